# revision 1
# baseline (speedup 1.0000x reference)
"""Single-head attention (B=4, L=4096, EMB=312, HID=256) on 8 NeuronCores.

Sharding: data-parallel over batch (4) x key-parallel (2) = 8 cores. Each
core handles ALL 4096 queries against its half of the keys and returns the
UNNORMALIZED partial [sum_k p*v | sum_k p] rows; the host combines the two
halves as (o1+o2)/(s1+s2). Key-sharding (vs query-sharding) halves the
duplicated K/V projection work; only the Q projection is duplicated.

Per-core device algorithm:
  - Host sends transposed, padded inputs split into bf16 (hi, lo) pairs; a
    matmul A@B is computed as A_hi@B_hi + A_lo@B_hi + A_hi@B_lo (the dropped
    lo@lo term is ~2^-18 relative), giving ~fp32-quality products at the
    bf16 PE rate (1 cycle/row).
  - embT carries a ones-row at index EMB and W* carry the bias in that row,
    so projections fold the bias in. Wv has 2 extra columns: ones (gives the
    softmax row-sum through the P@V matmul) and zero padding (even N).
  - Scores are computed transposed: sT[kl, ql] = kT-chunk^T @ qT, so the
    exp() output is directly the stationary operand for the P@V matmul —
    no on-device transposes anywhere.
  - Mask is host-side transposed, scaled by -1e5, cast to bf16; applied
    additively to the score PSUM by the vector engine. exp() on the scalar
    engine. The raw partials (P@V columns + row-sum column) go back to the
    host, which normalizes after combining the key-halves.

Precision ladder (BASS_KERNEL_PRECISION); projections are always bf16x2.
Measured absmax error relative to max|output| and TimelineSim time/core:
  - "fast":     fp32r single-pass QK and PV          ~9.4e-4 of scale
  - "fp16qk":   fp16 single-pass QK, fp32r PV        ~1.6e-3 of scale
  - "balanced": bf16x2 3-term QK, fp32r PV           ~1.6e-4, ~312 us  (default)
  - "exact":    also bf16x2 p/v in PV                ~3.8e-5, ~440 us
"""
import os

import numpy as np
import ml_dtypes

import concourse.bacc as bacc
import concourse.tile as tile
from concourse import mybir, bass2jax
from concourse.bass_utils import run_bass_kernel_spmd

# Debug aid (opt-in): surface real compile errors from the PJRT compile
# hook, which the C++ bridge otherwise swallows.
if os.environ.get("BASS_KERNEL_DEBUG"):
    import functools as _ft
    import traceback as _tb
    _orig_hook = bass2jax.neuronx_cc_hook
    @_ft.wraps(_orig_hook)
    def _dbg_hook(*args, **kwargs):
        try:
            return _orig_hook(*args, **kwargs)
        except BaseException:
            _tb.print_exc()
            raise
    bass2jax.neuronx_cc_hook = _dbg_hook

EMB, HID, B, L = 312, 256, 4, 4096
NCORES = 8
P = 128
KL = L // 2            # key rows per core (key-parallel halves)
EPAD = 384             # emb dim padded to 3 partition chunks; row EMB is the ones-row
HV = HID + 2           # v columns: HID values | ones | zero pad (even N for matmul)
QT = 512               # ql tile width (PSUM bank = 512 fp32)
NKC = KL // P          # 16 kl chunks per core
NQTT = L // QT         # 8 ql tiles per core (all queries)
NKT = KL // QT         # 4 l tiles for the k projection
MASK_SCALE = np.float32(-100000.0)

F32 = mybir.dt.float32
F16 = mybir.dt.float16
F32R = mybir.dt.float32r
BF16 = mybir.dt.bfloat16
BF = ml_dtypes.bfloat16

_CACHE = {}

# (lhs_piece, rhs_piece) index pairs for the 3-term bf16x2 product.
SPLIT3 = ((0, 0), (1, 0), (0, 1))


def _build(precision):
    qk_exact = precision in ("balanced", "exact")
    qk_fp16 = precision == "fp16qk"
    pv_exact = precision == "exact"

    nc = bacc.Bacc(None)

    def dram_pair(name, shape):
        return tuple(
            nc.dram_tensor(f"{name}{s}", shape, BF16, kind="ExternalInput")
            for s in ("_hi", "_lo")
        )

    embT = dram_pair("embT", [EPAD, L])
    embTk = dram_pair("embTk", [EPAD, KL])
    wq = dram_pair("wq", [EPAD, HID])
    wk = dram_pair("wk", [EPAD, HID])
    wv = dram_pair("wv", [EPAD, HV])
    maskT = nc.dram_tensor("maskT", [KL, L], BF16, kind="ExternalInput")
    out = nc.dram_tensor("out", [L, HID + 1], F32, kind="ExternalOutput")

    with tile.TileContext(nc) as tc:
        with (
            tc.tile_pool(name="big", bufs=1) as big,
            tc.tile_pool(name="wp", bufs=1) as wp,
            tc.tile_pool(name="mt", bufs=10) as mtp,
            tc.tile_pool(name="pt", bufs=4) as ptp,
            tc.tile_pool(name="fin", bufs=4) as fin,
            tc.tile_pool(name="ps_st", bufs=4, space="PSUM") as ps_st,
            tc.tile_pool(name="ps_pv", bufs=1, space="PSUM") as ps_pv,
        ):
            # ---- load inputs (as [P, chunk, free] with the chunk index in
            # the free dim; partition line p reads rows {p, 128+p, 256+p}).
            # Large tensors are loaded in column blocks, lowest columns first
            # across all chunks, so the first projection matmuls can start
            # ~2us in instead of waiting for the whole 6 MB transfer.
            def load_pair(pool, name, dram, ncol, blk=None):
                ts = [
                    pool.tile([P, 3, ncol], BF16, name=f"{name}_{s}", tag=f"{name}_{s}")
                    for s in ("hi", "lo")
                ]
                if blk is None:
                    # Small (weight) loads ride the second HWDGE ring (ACT)
                    # so they don't serialize ahead of the first embTk
                    # blocks on the SP ring at startup.
                    for t, d in zip(ts, dram):
                        nc.scalar.dma_start(out=t, in_=d[:, :].rearrange("(c p) n -> p c n", p=P))
                else:
                    # hi and lo interleaved per column block: the 3-term
                    # projection of block b needs both pieces of block b.
                    for b0 in range(0, ncol, blk):
                        for c in range(3):
                            for t, d in zip(ts, dram):
                                nc.sync.dma_start(
                                    out=t[:, c, b0:b0 + blk],
                                    in_=d[c * P:(c + 1) * P, b0:b0 + blk],
                                )
                return tuple(ts)

            wq_t = load_pair(wp, "wq", wq, HID)
            wk_t = load_pair(wp, "wk", wk, HID)
            wv_t = load_pair(wp, "wv", wv, HV)
            # embTk first: the projection phase starts with k/v tiles,
            # which consume the key-half slice.
            embTk_t = load_pair(big, "embTk", embTk, KL, blk=QT)
            embT_t = load_pair(big, "embT", embT, L, blk=QT)

            def mm3(ps, lhs_pair, rhs_pair, lslice, rslice):
                """ps = sum over 3 e-chunks of (lhs @ rhs) in bf16x2 3-term form."""
                n = len(SPLIT3) * 3
                i = 0
                for a, b in SPLIT3:
                    for e in range(3):
                        nc.tensor.matmul(
                            ps,
                            lhsT=lhs_pair[a][(slice(None), e) + lslice],
                            rhs=rhs_pair[b][(slice(None), e) + rslice],
                            start=(i == 0), stop=(i == n - 1),
                        )
                        i += 1

            # ---- projections
            # q/k in [h(part), hc, l(free)] layout; v in [kl(part), klc, h] layout.
            if qk_exact:
                kT_h = big.tile([P, 2, KL], BF16, name="kT_h")
                kT_l = big.tile([P, 2, KL], BF16, name="kT_l")
                qT_h = big.tile([P, 2, L], BF16, name="qT_h")
                qT_l = big.tile([P, 2, L], BF16, name="qT_l")
            elif qk_fp16:
                kT_r = big.tile([P, 2, KL], F16, name="kT_r")
                qT_r = big.tile([P, 2, L], F16, name="qT_r")
            else:
                kT_r = big.tile([P, 2, KL], F32R, name="kT_r")
                qT_r = big.tile([P, 2, L], F32R, name="qT_r")
            if pv_exact:
                v_h = big.tile([P, NKC, HV], BF16, name="v_h")
                v_l = big.tile([P, NKC, HV], BF16, name="v_l")
            else:
                v_r = big.tile([P, NKC, HV], F32R, name="v_r")

            def split_store(ps, hi_ap, lo_ap):
                nc.scalar.copy(out=hi_ap, in_=ps)
                nc.vector.tensor_sub(lo_ap, ps, hi_ap)

            def emit_kq(hc, lt, which):
                ps = ps_st.tile([P, QT], F32, name="st", tag="st")
                w, e, dsts = (
                    (wk_t, embTk_t, (kT_h, kT_l) if qk_exact else (kT_r,))
                    if which == "k"
                    else (wq_t, embT_t, (qT_h, qT_l) if qk_exact else (qT_r,))
                )
                mm3(ps, w, e, (slice(hc * P, (hc + 1) * P),),
                    (slice(lt * QT, (lt + 1) * QT),))
                dst = (slice(None), hc, slice(lt * QT, (lt + 1) * QT))
                if qk_exact:
                    split_store(ps, dsts[0][dst], dsts[1][dst])
                else:
                    nc.scalar.copy(out=dsts[0][dst], in_=ps)

            def emit_v(kc):
                ps = ps_st.tile([P, QT], F32, name="st", tag="st")
                mm3(ps[:, :HV], embTk_t, wv_t, (slice(kc * P, (kc + 1) * P),),
                    (slice(None),))
                dst = (slice(None), kc, slice(None))
                if pv_exact:
                    split_store(ps[:, :HV], v_h[dst], v_l[dst])
                else:
                    nc.scalar.copy(out=v_r[dst], in_=ps[:, :HV])

            # Interleave the k/q tiles (PSUM->SBUF copy has slack) with the
            # v tiles (copy-bound) so the scalar/vector copies never gate PE.
            kq_tiles = [("k", hc, lt) for hc in range(2) for lt in range(NKT)]
            kq_tiles += [("q", hc, lt) for hc in range(2) for lt in range(NQTT)]
            vi = 0
            for i, (which, hc, lt) in enumerate(kq_tiles):
                emit_kq(hc, lt, which)
                want_v = ((i + 1) * NKC) // len(kq_tiles)
                while vi < want_v:
                    emit_v(vi)
                    vi += 1
            while vi < NKC:
                emit_v(vi)
                vi += 1

            # ---- attention
            # Software-pipelined emission: chunk kc's P@V matmuls are emitted
            # AFTER chunk kc+1's QK matmuls, so the PE always has independent
            # work in program order while the DVE mask-add + ACT exp of the
            # current chunk are still in flight.
            for qt in range(NQTT):
                pvs = [
                    ps_pv.tile([P, HV], F32, name=f"pv{j}", tag=f"pv{j}")
                    for j in range(4)
                ]
                qsl = slice(qt * QT, (qt + 1) * QT)
                pending_pv = None  # (kc, p-tiles) awaiting PV emission

                def emit_pv(kc, ptile):
                    for j in range(4):
                        jsl = slice(j * P, (j + 1) * P)
                        if pv_exact:
                            for t, (a, b) in enumerate(SPLIT3):
                                nc.tensor.matmul(
                                    pvs[j],
                                    lhsT=ptile[a][:, jsl],
                                    rhs=(v_h, v_l)[b][:, kc, :],
                                    start=(kc == 0 and t == 0),
                                    stop=(kc == NKC - 1 and t == 2),
                                )
                        else:
                            nc.tensor.matmul(
                                pvs[j],
                                lhsT=ptile[:, jsl],
                                rhs=v_r[:, kc, :],
                                start=(kc == 0), stop=(kc == NKC - 1),
                            )

                for kc in range(NKC):
                    ksl = slice(kc * P, (kc + 1) * P)
                    st = ps_st.tile([P, QT], F32, name="st", tag="st")
                    if qk_exact:
                        kp, qp = (kT_h, kT_l), (qT_h, qT_l)
                        n = 2 * len(SPLIT3)
                        i = 0
                        for a, b in SPLIT3:
                            for hc in range(2):
                                nc.tensor.matmul(
                                    st,
                                    lhsT=kp[a][:, hc, ksl],
                                    rhs=qp[b][:, hc, qsl],
                                    start=(i == 0), stop=(i == n - 1),
                                )
                                i += 1
                    else:
                        for hc in range(2):
                            nc.tensor.matmul(
                                st,
                                lhsT=kT_r[:, hc, ksl],
                                rhs=qT_r[:, hc, qsl],
                                start=(hc == 0), stop=(hc == 1),
                            )
                    if pending_pv is not None:
                        emit_pv(*pending_pv)
                    mt = mtp.tile([P, QT], BF16, name="mt", tag="mt")
                    nc.sync.dma_start(out=mt, in_=maskT[ksl, qsl])
                    nc.vector.tensor_tensor(out=st, in0=st, in1=mt, op=mybir.AluOpType.add)
                    if pv_exact:
                        pe = ptp.tile([P, QT], F32, name="pe", tag="pe")
                        nc.scalar.activation(out=pe, in_=st, func=mybir.ActivationFunctionType.Exp)
                        p_h = ptp.tile([P, QT], BF16, name="p_h", tag="p_h")
                        p_l = ptp.tile([P, QT], BF16, name="p_l", tag="p_l")
                        nc.vector.tensor_copy(p_h, pe)
                        nc.gpsimd.tensor_sub(p_l, pe, p_h)
                        pending_pv = (kc, (p_h, p_l))
                    else:
                        pt = ptp.tile([P, QT], F32R, name="pt", tag="pt")
                        nc.scalar.activation(out=pt, in_=st, func=mybir.ActivationFunctionType.Exp)
                        pending_pv = (kc, pt)
                emit_pv(*pending_pv)
                for j in range(4):
                    # Ship the unnormalized partial [sum p*v | sum p]; the
                    # host divides after combining the two key-halves.
                    ot = fin.tile([P, HID + 1], F32, name="ot", tag="ot")
                    nc.vector.tensor_copy(ot, pvs[j][:, :HID + 1])
                    row0 = (qt * 4 + j) * P
                    nc.sync.dma_start(out=out[row0:row0 + P, :], in_=ot)
    nc.finalize()
    return nc


def _get_nc():
    precision = os.environ.get("BASS_KERNEL_PRECISION", "balanced")
    key = f"nc_{precision}"
    if key not in _CACHE:
        _CACHE[key] = _build(precision)
    return _CACHE[key]


def _split_pair(x):
    hi = x.astype(BF)
    lo = (x - hi.astype(np.float32)).astype(BF)
    return hi, lo


def kernel(embedding, mask, Wq, bq, Wk, bk, Wv, bv):
    embedding = np.asarray(embedding, dtype=np.float32)
    mask = np.asarray(mask, dtype=np.float32)
    Wq = np.asarray(Wq, dtype=np.float32)
    Wk = np.asarray(Wk, dtype=np.float32)
    Wv = np.asarray(Wv, dtype=np.float32)
    bq = np.asarray(bq, dtype=np.float32)
    bk = np.asarray(bk, dtype=np.float32)
    bv = np.asarray(bv, dtype=np.float32)

    def pad_w(w, b, extra_one=False):
        wp = np.zeros((EPAD, HV if extra_one else HID), dtype=np.float32)
        wp[:EMB, :HID] = w
        wp[EMB, :HID] = b
        if extra_one:
            wp[EMB, HID] = 1.0
        return wp

    wq_h, wq_l = _split_pair(pad_w(Wq, bq))
    wk_h, wk_l = _split_pair(pad_w(Wk, bk))
    wv_h, wv_l = _split_pair(pad_w(Wv, bv, extra_one=True))

    in_maps = []
    for c in range(NCORES):
        b, half = divmod(c, 2)
        embT = np.zeros((EPAD, L), dtype=np.float32)
        embT[:EMB] = embedding[b].T
        embT[EMB] = 1.0
        e_h, e_l = _split_pair(embT)
        ksl = slice(half * KL, (half + 1) * KL)
        ek_h = np.ascontiguousarray(e_h[:, ksl])
        ek_l = np.ascontiguousarray(e_l[:, ksl])
        mT = np.ascontiguousarray(mask[b].T[ksl, :])
        mT = (mT * MASK_SCALE).astype(BF)
        in_maps.append({
            "embT_hi": e_h, "embT_lo": e_l,
            "embTk_hi": ek_h, "embTk_lo": ek_l,
            "wq_hi": wq_h, "wq_lo": wq_l,
            "wk_hi": wk_h, "wk_lo": wk_l,
            "wv_hi": wv_h, "wv_lo": wv_l,
            "maskT": mT,
        })

    nc = _get_nc()
    trace = bool(int(os.environ.get("BASS_KERNEL_TRACE", "0")))
    res = run_bass_kernel_spmd(nc, in_maps, core_ids=list(range(NCORES)), trace=trace)
    _CACHE["last_results"] = res

    full = np.empty((B, L, HID), dtype=np.float32)
    for b in range(B):
        r0 = res.results[2 * b]["out"].astype(np.float64)
        r1 = res.results[2 * b + 1]["out"].astype(np.float64)
        num = r0[:, :HID] + r1[:, :HID]
        den = r0[:, HID:] + r1[:, HID:]
        full[b] = (num / den).astype(np.float32)
    return full



# revision 5
# speedup vs baseline: 1.8475x; 1.8475x over previous
"""Single-head attention (B=4, L=4096, EMB=312, HID=256) on 8 NeuronCores.

Sharding: data-parallel over batch (4) x key-parallel (2) = 8 cores. Each
core handles ALL 4096 queries against its half of the keys and returns the
UNNORMALIZED partial [sum_k p*v | sum_k p] rows; the host combines the two
halves as (o1+o2)/(s1+s2).

Per-core device algorithm ("turbo"):
  - Every matmul is a single-pass fp32r op (1 row/cycle on the PE with a
    >=256-wide moving dim, near-fp32 accuracy). No hi/lo splits anywhere:
    projections contract 3 e-chunks, QK contracts 2 h-chunks, PV contracts
    16 key-chunks, all accumulating in PSUM.
  - The host rotates the embedding's sequence axis per-core so that the
    core's OWN key half sits in columns 0..2047; k/v projections then read
    a prefix of the same SBUF tile the q projection uses (no duplicate
    embTk load). Output rows come back in rotated order; the host unrolls.
  - embT carries a ones-row at index EMB and W* carry the bias in that row,
    so projections fold the bias in. Wv has 2 extra columns: ones (gives the
    softmax row-sum through the P@V matmul) and zero padding (even N).
  - Scores are computed transposed: sT[kl, ql] = kT-chunk^T @ qT, so the
    exp() output is directly the stationary operand for the P@V matmul -
    no on-device transposes anywhere.
  - Mask is host-side transposed/rotated, scaled to a large negative, cast
    to bf16; applied additively to the score PSUM by the vector engine
    (one batched DMA per 512-query tile). exp() on the scalar engine.
    PSUM-reading copies split across DVE (k/v) and ACT (q, output); the
    gpsimd engine has no PSUM port. The raw partials go back to the host,
    which normalizes after combining the key-halves.
  - All input DMAs are issued on the SP ring in dependency order so the
    (serialized) DMA engines deliver blocks just-in-time for the PE.
"""
import os

import numpy as np
import ml_dtypes

import concourse.bacc as bacc
import concourse.tile as tile
from concourse import mybir, bass2jax
from concourse.bass_utils import run_bass_kernel_spmd

# Debug aid (opt-in): surface real compile errors from the PJRT compile
# hook, which the C++ bridge otherwise swallows.
if os.environ.get("BASS_KERNEL_DEBUG"):
    import functools as _ft
    import traceback as _tb
    _orig_hook = bass2jax.neuronx_cc_hook
    @_ft.wraps(_orig_hook)
    def _dbg_hook(*args, **kwargs):
        try:
            return _orig_hook(*args, **kwargs)
        except BaseException:
            _tb.print_exc()
            raise
    bass2jax.neuronx_cc_hook = _dbg_hook

EMB, HID, B, L = 312, 256, 4, 4096
NCORES = 8
P = 128
KL = L // 2            # key rows per core (key-parallel halves)
EPAD = 384             # emb dim padded to 3 partition chunks; row EMB is the ones-row
HV = HID + 2           # v columns: HID values | ones | zero pad (even N)
QT = 512               # ql tile width (PSUM bank = 512 fp32)
NKC = KL // P          # 16 kl chunks per core
NQT = L // QT          # 8 ql tiles per core (all queries)
NKT = KL // QT         # 4 l tiles for the k projection
NEB = L // QT          # 8 emb column blocks
MASK_SCALE = np.float32(-100000.0)

F32 = mybir.dt.float32
F32R = mybir.dt.float32r
BF16 = mybir.dt.bfloat16
BF = ml_dtypes.bfloat16

_CACHE = {}


def _build(precision="turbo"):
    nc = bacc.Bacc(None)

    embT = nc.dram_tensor("embT", [EPAD, L], F32R, kind="ExternalInput")
    wq = nc.dram_tensor("wq", [EPAD, HID], F32R, kind="ExternalInput")
    wk = nc.dram_tensor("wk", [EPAD, HID], F32R, kind="ExternalInput")
    wv = nc.dram_tensor("wv", [EPAD, HV], F32R, kind="ExternalInput")
    maskT = nc.dram_tensor("maskT", [KL, L], BF16, kind="ExternalInput")
    out = nc.dram_tensor("out", [L, HID + 1], F32, kind="ExternalOutput")

    with tile.TileContext(nc) as tc:
        with (
            tc.tile_pool(name="big", bufs=1) as big,
            tc.tile_pool(name="mt", bufs=2) as mtp,
            tc.tile_pool(name="pt", bufs=4) as ptp,
            tc.tile_pool(name="fin", bufs=4) as fin,
            tc.tile_pool(name="ps_st", bufs=4, space="PSUM") as ps_st,
            tc.tile_pool(name="ps_pv", bufs=1, space="PSUM") as ps_pv,
        ):
            emb_t = big.tile([P, 3, L], F32R, name="emb")
            wq_t = big.tile([P, 3, HID], F32R, name="wq")
            wk_t = big.tile([P, 3, HID], F32R, name="wk")
            wv_t = big.tile([P, 3, HV], F32R, name="wv")
            qT = big.tile([P, 2, L], F32R, name="qT")
            kT = big.tile([P, 2, KL], F32R, name="kT")
            v_r = big.tile([P, NKC, HV], F32R, name="v_r")

            # ---- DMA plan (all on the SP ring: issue order == transfer
            # order on the serialized DMA engines). Weights + the emb block
            # each projection tile needs arrive just ahead of the PE.
            def dma_w(t, d):
                nc.sync.dma_start(out=t, in_=d[:, :].rearrange("(c p) n -> p c n", p=P))

            def dma_emb(blk):
                sl = slice(blk * QT, (blk + 1) * QT)
                nc.sync.dma_start(
                    out=emb_t[:, :, sl],
                    in_=embT[:, sl].rearrange("(c p) n -> p c n", p=P),
                )

            mts = [None] * NQT

            def dma_mask(qt):
                mts[qt] = mtp.tile([P, NKC, QT], BF16, name="mt", tag="mt")
                qsl = slice(qt * QT, (qt + 1) * QT)
                nc.sync.dma_start(
                    out=mts[qt],
                    in_=maskT[:, qsl].rearrange("(c p) q -> p c q", p=P),
                )

            dma_w(wk_t, wk)
            dma_w(wv_t, wv)
            dma_emb(0)
            dma_mask(0)
            dma_emb(1)
            dma_w(wq_t, wq)
            dma_emb(2)
            dma_mask(1)
            for blk in range(3, NEB):
                dma_emb(blk)

            # ---- projections (single-pass fp32r, 3 e-chunk contraction)
            # q/k in [h(part), hc, l(free)] layout; v in [kl(part), klc, h].
            def emit_kq(hc, lt, which):
                ps = ps_st.tile([P, QT], F32, name="st", tag="st")
                w, dst = (wk_t, kT) if which == "k" else (wq_t, qT)
                hsl = slice(hc * P, (hc + 1) * P)
                lsl = slice(lt * QT, (lt + 1) * QT)
                for e in range(3):
                    nc.tensor.matmul(
                        ps,
                        lhsT=w[:, e, hsl],
                        rhs=emb_t[:, e, lsl],
                        start=(e == 0), stop=(e == 2),
                    )
                # k copies on DVE, q copies on ACT (balance the engines).
                if which == "k":
                    nc.vector.tensor_copy(dst[:, hc, lsl], ps)
                else:
                    nc.scalar.copy(out=dst[:, hc, lsl], in_=ps)

            def emit_v(kc):
                ps = ps_st.tile([P, QT], F32, name="st", tag="st")
                ksl = slice(kc * P, (kc + 1) * P)
                for e in range(3):
                    nc.tensor.matmul(
                        ps[:, :HV],
                        lhsT=emb_t[:, e, ksl],
                        rhs=wv_t[:, e, :],
                        start=(e == 0), stop=(e == 2),
                    )
                nc.vector.tensor_copy(v_r[:, kc, :], ps[:, :HV])

            for blk in range(4):
                emit_kq(0, blk, "k")
                emit_kq(1, blk, "k")
                for i in range(4):
                    emit_v(4 * blk + i)
                emit_kq(0, blk, "q")
                emit_kq(1, blk, "q")
            for blk in range(4, NEB):
                emit_kq(0, blk, "q")
                emit_kq(1, blk, "q")

            # ---- attention
            # Software-pipelined emission: chunk kc's P@V matmuls are emitted
            # AFTER chunk kc+1's QK matmuls, so the PE always has independent
            # work in program order while the DVE mask-add + ACT exp of the
            # current chunk are still in flight.
            for qt in range(NQT):
                if qt + 2 < NQT:
                    dma_mask(qt + 2)
                mt = mts[qt]
                pvs = [
                    ps_pv.tile([P, HV], F32, name=f"pv{j}", tag=f"pv{j}")
                    for j in range(4)
                ]
                qsl = slice(qt * QT, (qt + 1) * QT)
                pending_pv = None  # (kc, p-tile) awaiting PV emission

                def emit_pv(kc, ptile):
                    for j in range(4):
                        jsl = slice(j * P, (j + 1) * P)
                        nc.tensor.matmul(
                            pvs[j],
                            lhsT=ptile[:, jsl],
                            rhs=v_r[:, kc, :],
                            start=(kc == 0), stop=(kc == NKC - 1),
                        )

                for kc in range(NKC):
                    ksl = slice(kc * P, (kc + 1) * P)
                    st = ps_st.tile([P, QT], F32, name="st", tag="st")
                    for hc in range(2):
                        nc.tensor.matmul(
                            st,
                            lhsT=kT[:, hc, ksl],
                            rhs=qT[:, hc, qsl],
                            start=(hc == 0), stop=(hc == 1),
                        )
                    if pending_pv is not None:
                        emit_pv(*pending_pv)
                    nc.vector.tensor_tensor(out=st, in0=st, in1=mt[:, kc, :], op=mybir.AluOpType.add)
                    pt = ptp.tile([P, QT], F32R, name="pt", tag="pt")
                    nc.scalar.activation(out=pt, in_=st, func=mybir.ActivationFunctionType.Exp)
                    pending_pv = (kc, pt)
                emit_pv(*pending_pv)
                for j in range(4):
                    # Ship the unnormalized partial [sum p*v | sum p]; the
                    # host divides after combining the two key-halves.
                    ot = fin.tile([P, HID + 1], F32, name="ot", tag="ot")
                    nc.scalar.copy(out=ot, in_=pvs[j][:, :HID + 1])
                    row0 = (qt * 4 + j) * P
                    nc.sync.dma_start(out=out[row0:row0 + P, :], in_=ot)
    nc.finalize()
    return nc


def _get_nc():
    precision = os.environ.get("BASS_KERNEL_PRECISION", "turbo")
    key = f"nc_{precision}"
    if key not in _CACHE:
        _CACHE[key] = _build(precision)
    return _CACHE[key]


def kernel(embedding, mask, Wq, bq, Wk, bk, Wv, bv):
    embedding = np.asarray(embedding, dtype=np.float32)
    mask = np.asarray(mask, dtype=np.float32)
    Wq = np.asarray(Wq, dtype=np.float32)
    Wk = np.asarray(Wk, dtype=np.float32)
    Wv = np.asarray(Wv, dtype=np.float32)
    bq = np.asarray(bq, dtype=np.float32)
    bk = np.asarray(bk, dtype=np.float32)
    bv = np.asarray(bv, dtype=np.float32)

    def pad_w(w, b, extra_one=False):
        wp = np.zeros((EPAD, HV if extra_one else HID), dtype=np.float32)
        wp[:EMB, :HID] = w
        wp[EMB, :HID] = b
        if extra_one:
            wp[EMB, HID] = 1.0
        return wp

    wq_p = pad_w(Wq, bq)
    wk_p = pad_w(Wk, bk)
    wv_p = pad_w(Wv, bv, extra_one=True)

    in_maps = []
    for c in range(NCORES):
        b, half = divmod(c, 2)
        # Rotate the sequence so this core's key half is rows 0..KL-1.
        emb_r = np.roll(embedding[b], -half * KL, axis=0)
        eT = np.zeros((EPAD, L), dtype=np.float32)
        eT[:EMB] = emb_r.T
        eT[EMB] = 1.0
        # maskT rows: this core's keys (original order); cols: rotated q.
        mT = np.roll(mask[b], -half * KL, axis=0)[:, half * KL:(half + 1) * KL].T
        mT = np.ascontiguousarray(mT * MASK_SCALE).astype(BF)
        in_maps.append({
            "embT": eT,
            "wq": wq_p, "wk": wk_p, "wv": wv_p,
            "maskT": mT,
        })

    nc = _get_nc()
    trace = bool(int(os.environ.get("BASS_KERNEL_TRACE", "0")))
    res = run_bass_kernel_spmd(nc, in_maps, core_ids=list(range(NCORES)), trace=trace)
    _CACHE["last_results"] = res

    full = np.empty((B, L, HID), dtype=np.float32)
    for b in range(B):
        r0 = res.results[2 * b]["out"].astype(np.float64)
        r1 = res.results[2 * b + 1]["out"].astype(np.float64)
        # r1 rows are in rotated q order (q = row + KL mod L); unroll.
        r1 = np.roll(r1, KL, axis=0)
        num = r0[:, :HID] + r1[:, :HID]
        den = r0[:, HID:] + r1[:, HID:]
        full[b] = (num / den).astype(np.float32)
    return full


# revision 6
# speedup vs baseline: 2.1178x; 1.1463x over previous
"""Single-head attention (B=4, L=4096, EMB=312, HID=256) on 8 NeuronCores.

Sharding: data-parallel over batch (4) x key-parallel (2) = 8 cores. Each
core handles ALL 4096 queries against its half of the keys and returns the
UNNORMALIZED partial [sum_k p*v | sum_k p] rows; the host combines the two
halves as (o1+o2)/(s1+s2).

Per-core device algorithm ("turbo"):
  - Every matmul is a single-pass fp32r op (1 row/cycle on the PE with a
    >=256-wide moving dim, near-fp32 accuracy). No hi/lo splits anywhere:
    projections contract 3 e-chunks, QK contracts 2 h-chunks, PV contracts
    16 key-chunks, all accumulating in PSUM.
  - The host rotates the embedding's sequence axis per-core so that the
    core's OWN key half sits in columns 0..2047; k/v projections then read
    a prefix of the same SBUF tile the q projection uses (no duplicate
    embTk load). Output rows come back in rotated order; the host unrolls.
  - embT carries a ones-row at index EMB and W* carry the bias in that row,
    so projections fold the bias in. Wv has 2 extra columns: ones (gives the
    softmax row-sum through the P@V matmul) and zero padding (even N).
  - Scores are computed transposed: sT[kl, ql] = kT-chunk^T @ qT, so the
    exp() output is directly the stationary operand for the P@V matmul -
    no on-device transposes anywhere.
  - Mask is host-side transposed/rotated, scaled to a large negative, cast
    to bf16; applied additively to the score PSUM by the vector engine
    (one batched DMA per 512-query tile). exp() on the scalar engine.
    PSUM-reading copies are split across DVE (k/v, half the outputs) and
    ACT (q, the other half); gpsimd has no PSUM port.
  - The attention is a single flat software pipeline over (qt, kc) chunks:
    chunk t's P@V matmuls are emitted LAG chunks later, so the PE always
    has independent work in program order while the DVE mask-add + ACT exp
    of recent chunks are in flight. The pipeline runs straight across qt
    boundaries; the q-projection tiles for late query blocks are interleaved
    into early attention chunks as extra PE gap-fillers while their emb
    blocks stream in.
  - All input DMAs ride the SP ring in a hand-ordered sequence so the
    (serialized) DMA engines deliver each block just ahead of first use.
"""
import os

import numpy as np
import ml_dtypes

import concourse.bacc as bacc
import concourse.tile as tile
from concourse import mybir, bass2jax
from concourse.bass_utils import run_bass_kernel_spmd

# Debug aid (opt-in): surface real compile errors from the PJRT compile
# hook, which the C++ bridge otherwise swallows.
if os.environ.get("BASS_KERNEL_DEBUG"):
    import functools as _ft
    import traceback as _tb
    _orig_hook = bass2jax.neuronx_cc_hook
    @_ft.wraps(_orig_hook)
    def _dbg_hook(*args, **kwargs):
        try:
            return _orig_hook(*args, **kwargs)
        except BaseException:
            _tb.print_exc()
            raise
    bass2jax.neuronx_cc_hook = _dbg_hook

EMB, HID, B, L = 312, 256, 4, 4096
NCORES = 8
P = 128
KL = L // 2            # key rows per core (key-parallel halves)
EPAD = 384             # emb dim padded to 3 partition chunks; row EMB is the ones-row
HV = HID + 2           # v columns: HID values | ones | zero pad (even N)
QT = 512               # ql tile width (PSUM bank = 512 fp32)
NKC = KL // P          # 16 kl chunks per core
NQT = L // QT          # 8 ql tiles per core (all queries)
NEB = L // QT          # 8 emb column blocks
LAG = 3                # attention pipeline depth, in (qt, kc) chunks
MASK_SCALE = np.float32(-100000.0)

F32 = mybir.dt.float32
F32R = mybir.dt.float32r
BF16 = mybir.dt.bfloat16
BF = ml_dtypes.bfloat16

_CACHE = {}

# Leftover q-projection tiles (hc, block) interleaved into attention chunks
# (qt, kc) while their emb blocks are still streaming in.
_Q_INSERTS = {
    (0, 8): (0, 4), (0, 12): (1, 4),
    (1, 0): (0, 5), (1, 4): (1, 5), (1, 8): (0, 6), (1, 12): (1, 6),
    (2, 0): (0, 7), (2, 4): (1, 7),
}


def _build(precision="turbo"):
    nc = bacc.Bacc(None)

    embT = nc.dram_tensor("embT", [EPAD, L], F32R, kind="ExternalInput")
    wq = nc.dram_tensor("wq", [EPAD, HID], F32R, kind="ExternalInput")
    wk = nc.dram_tensor("wk", [EPAD, HID], F32R, kind="ExternalInput")
    wv = nc.dram_tensor("wv", [EPAD, HV], F32R, kind="ExternalInput")
    maskT = nc.dram_tensor("maskT", [KL, L], BF16, kind="ExternalInput")
    out = nc.dram_tensor("out", [L, HID + 1], F32, kind="ExternalOutput")

    with tile.TileContext(nc) as tc:
        with (
            tc.tile_pool(name="big", bufs=1) as big,
            tc.tile_pool(name="mt", bufs=3) as mtp,
            tc.tile_pool(name="pt", bufs=4) as ptp,
            tc.tile_pool(name="fin", bufs=2) as fin,
            tc.tile_pool(name="ps_st", bufs=4, space="PSUM") as ps_st,
            tc.tile_pool(name="ps_pv", bufs=1, space="PSUM") as ps_pv,
        ):
            emb_t = big.tile([P, 3, L], F32R, name="emb")
            wq_t = big.tile([P, 3, HID], F32R, name="wq")
            wk_t = big.tile([P, 3, HID], F32R, name="wk")
            wv_t = big.tile([P, 3, HV], F32R, name="wv")
            qT = big.tile([P, 2, L], F32R, name="qT")
            kT = big.tile([P, 2, KL], F32R, name="kT")
            v_r = big.tile([P, NKC, HV], F32R, name="v_r")

            # ---- DMA plan (all on the SP ring: issue order == transfer
            # order on the serialized DMA engines). Hand-ordered so each
            # block lands just ahead of its first PE use.
            def dma_cols(t, d, c0, c1):
                nc.sync.dma_start(
                    out=t[:, :, c0:c1],
                    in_=d[:, c0:c1].rearrange("(c p) n -> p c n", p=P),
                )

            def dma_emb(blk):
                dma_cols(emb_t, embT, blk * QT, (blk + 1) * QT)

            mts = [None] * NQT

            def dma_mask(qt):
                mts[qt] = mtp.tile([P, NKC, QT], BF16, name="mt", tag="mt")
                qsl = slice(qt * QT, (qt + 1) * QT)
                nc.sync.dma_start(
                    out=mts[qt],
                    in_=maskT[:, qsl].rearrange("(c p) q -> p c q", p=P),
                )

            dma_cols(wk_t, wk, 0, P)        # first k-proj tile's stationary
            dma_cols(emb_t, embT, 0, 256)   # first half of block 0
            dma_cols(wk_t, wk, P, HID)
            dma_cols(emb_t, embT, 256, 512)
            dma_cols(wv_t, wv, 0, HV)
            dma_emb(1)
            dma_cols(wq_t, wq, 0, HID)
            dma_emb(2)
            dma_emb(3)
            dma_mask(0)
            dma_emb(4)
            dma_mask(1)
            dma_emb(5)
            dma_emb(6)
            dma_mask(2)
            dma_emb(7)

            # ---- projections (single-pass fp32r, 3 e-chunk contraction)
            # q/k in [h(part), hc, l(free)] layout; v in [kl(part), klc, h].
            def emit_kq(hc, c0, c1, which):
                ps = ps_st.tile([P, QT], F32, name="st", tag="st")
                w, dst = (wk_t, kT) if which == "k" else (wq_t, qT)
                hsl = slice(hc * P, (hc + 1) * P)
                for e in range(3):
                    nc.tensor.matmul(
                        ps[:, :c1 - c0],
                        lhsT=w[:, e, hsl],
                        rhs=emb_t[:, e, c0:c1],
                        start=(e == 0), stop=(e == 2),
                    )
                # k copies on DVE, q copies on ACT (balance the engines).
                if which == "k":
                    nc.vector.tensor_copy(dst[:, hc, c0:c1], ps[:, :c1 - c0])
                else:
                    nc.scalar.copy(out=dst[:, hc, c0:c1], in_=ps[:, :c1 - c0])

            def emit_q(hc, blk):
                emit_kq(hc, blk * QT, (blk + 1) * QT, "q")

            def emit_v(kc):
                ps = ps_st.tile([P, QT], F32, name="st", tag="st")
                ksl = slice(kc * P, (kc + 1) * P)
                for e in range(3):
                    nc.tensor.matmul(
                        ps[:, :HV],
                        lhsT=emb_t[:, e, ksl],
                        rhs=wv_t[:, e, :],
                        start=(e == 0), stop=(e == 2),
                    )
                nc.vector.tensor_copy(v_r[:, kc, :], ps[:, :HV])

            # Block 0 in quarter-steps (its DMAs are split for fast start),
            # then blocks 1..3 with the early q tiles woven in. q tiles for
            # blocks 4..7 ride inside the attention stream (_Q_INSERTS).
            emit_kq(0, 0, 256, "k")
            emit_kq(1, 0, 256, "k")
            emit_v(0)
            emit_v(1)
            emit_kq(0, 256, 512, "k")
            emit_kq(1, 256, 512, "k")
            emit_v(2)
            emit_v(3)
            for blk in range(1, 4):
                emit_kq(0, blk * QT, (blk + 1) * QT, "k")
                emit_kq(1, blk * QT, (blk + 1) * QT, "k")
                for i in range(4):
                    emit_v(4 * blk + i)
                emit_q(0, blk - 1)
                emit_q(1, blk - 1)
            emit_q(0, 3)
            emit_q(1, 3)

            # ---- attention: flat pipeline over 128 (qt, kc) chunks.
            pvs = None
            pend = []  # chunks whose P@V emission is deferred by LAG

            def flush_pv():
                qt, kc, pt, pv = pend.pop(0)
                for j in range(4):
                    jsl = slice(j * P, (j + 1) * P)
                    nc.tensor.matmul(
                        pv[j],
                        lhsT=pt[:, jsl],
                        rhs=v_r[:, kc, :],
                        start=(kc == 0), stop=(kc == NKC - 1),
                    )
                if kc == NKC - 1:
                    # Ship the unnormalized partial [sum p*v | sum p]; the
                    # host divides after combining the two key-halves. Copies
                    # split DVE/ACT; one batched out-DMA per qt on SP.
                    ot = fin.tile([P, 4, HID + 1], F32, name="ot", tag="ot")
                    for j in range(4):
                        src = pv[j][:, :HID + 1]
                        if j < 2:
                            nc.vector.tensor_copy(ot[:, j, :], src)
                        else:
                            nc.scalar.copy(out=ot[:, j, :], in_=src)
                    r0 = qt * QT
                    nc.sync.dma_start(
                        out=out[r0:r0 + QT, :].rearrange("(j p) h -> p j h", p=P),
                        in_=ot,
                    )

            for qt in range(NQT):
                if qt + 3 < NQT:
                    dma_mask(qt + 3)
                mt = mts[qt]
                pvs = [
                    ps_pv.tile([P, HV], F32, name=f"pv{j}", tag=f"pv{j}")
                    for j in range(4)
                ]
                qsl = slice(qt * QT, (qt + 1) * QT)
                for kc in range(NKC):
                    qi = _Q_INSERTS.get((qt, kc))
                    if qi is not None:
                        emit_q(*qi)
                    ksl = slice(kc * P, (kc + 1) * P)
                    st = ps_st.tile([P, QT], F32, name="st", tag="st")
                    for hc in range(2):
                        nc.tensor.matmul(
                            st,
                            lhsT=kT[:, hc, ksl],
                            rhs=qT[:, hc, qsl],
                            start=(hc == 0), stop=(hc == 1),
                        )
                    if len(pend) >= LAG:
                        flush_pv()
                    nc.vector.tensor_tensor(out=st, in0=st, in1=mt[:, kc, :], op=mybir.AluOpType.add)
                    pt = ptp.tile([P, QT], F32R, name="pt", tag="pt")
                    nc.scalar.activation(out=pt, in_=st, func=mybir.ActivationFunctionType.Exp)
                    pend.append((qt, kc, pt, pvs))
            while pend:
                flush_pv()
    nc.finalize()
    return nc


def _get_nc():
    precision = os.environ.get("BASS_KERNEL_PRECISION", "turbo")
    key = f"nc_{precision}"
    if key not in _CACHE:
        _CACHE[key] = _build(precision)
    return _CACHE[key]


def kernel(embedding, mask, Wq, bq, Wk, bk, Wv, bv):
    embedding = np.asarray(embedding, dtype=np.float32)
    mask = np.asarray(mask, dtype=np.float32)
    Wq = np.asarray(Wq, dtype=np.float32)
    Wk = np.asarray(Wk, dtype=np.float32)
    Wv = np.asarray(Wv, dtype=np.float32)
    bq = np.asarray(bq, dtype=np.float32)
    bk = np.asarray(bk, dtype=np.float32)
    bv = np.asarray(bv, dtype=np.float32)

    def pad_w(w, b, extra_one=False):
        wp = np.zeros((EPAD, HV if extra_one else HID), dtype=np.float32)
        wp[:EMB, :HID] = w
        wp[EMB, :HID] = b
        if extra_one:
            wp[EMB, HID] = 1.0
        return wp

    wq_p = pad_w(Wq, bq)
    wk_p = pad_w(Wk, bk)
    wv_p = pad_w(Wv, bv, extra_one=True)

    in_maps = []
    for c in range(NCORES):
        b, half = divmod(c, 2)
        # Rotate the sequence so this core's key half is rows 0..KL-1.
        emb_r = np.roll(embedding[b], -half * KL, axis=0)
        eT = np.zeros((EPAD, L), dtype=np.float32)
        eT[:EMB] = emb_r.T
        eT[EMB] = 1.0
        # maskT rows: this core's keys (original order); cols: rotated q.
        mT = np.roll(mask[b], -half * KL, axis=0)[:, half * KL:(half + 1) * KL].T
        mT = np.ascontiguousarray(mT * MASK_SCALE).astype(BF)
        in_maps.append({
            "embT": eT,
            "wq": wq_p, "wk": wk_p, "wv": wv_p,
            "maskT": mT,
        })

    nc = _get_nc()
    trace = bool(int(os.environ.get("BASS_KERNEL_TRACE", "0")))
    res = run_bass_kernel_spmd(nc, in_maps, core_ids=list(range(NCORES)), trace=trace)
    _CACHE["last_results"] = res

    full = np.empty((B, L, HID), dtype=np.float32)
    for b in range(B):
        r0 = res.results[2 * b]["out"].astype(np.float64)
        r1 = res.results[2 * b + 1]["out"].astype(np.float64)
        # r1 rows are in rotated q order (q = row + KL mod L); unroll.
        r1 = np.roll(r1, KL, axis=0)
        num = r0[:, :HID] + r1[:, :HID]
        den = r0[:, HID:] + r1[:, HID:]
        full[b] = (num / den).astype(np.float32)
    return full


# revision 10
# speedup vs baseline: 2.1372x; 1.0092x over previous
"""Single-head attention (B=4, L=4096, EMB=312, HID=256) on 8 NeuronCores.

Sharding: data-parallel over batch (4) x key-parallel (2) = 8 cores. Each
core handles ALL 4096 queries against its half of the keys and returns the
UNNORMALIZED partial [sum_k p*v | sum_k p] rows; the host combines the two
halves as (o1+o2)/(s1+s2).

Per-core device algorithm ("turbo"):
  - Every matmul is a single-pass fp32r op (1 row/cycle on the PE with a
    >=256-wide moving dim, near-fp32 accuracy). No hi/lo splits anywhere:
    projections contract 3 e-chunks, QK contracts 2 h-chunks, PV contracts
    16 key-chunks, all accumulating in PSUM.
  - The host rotates the embedding's sequence axis per-core so that the
    core's OWN key half sits in columns 0..2047; k/v projections then read
    a prefix of the same SBUF tile the q projection uses (no duplicate
    embTk load). Output rows come back in rotated order; the host unrolls.
  - embT carries a ones-row at index EMB and W* carry the bias in that row,
    so projections fold the bias in. Wv has 2 extra columns: ones (gives the
    softmax row-sum through the P@V matmul) and zero padding (even N).
  - Scores are computed transposed: sT[kl, ql] = kT-chunk^T @ qT, so the
    exp() output is directly the stationary operand for the P@V matmul -
    no on-device transposes anywhere.
  - Mask is host-side transposed/rotated, scaled to a large negative, cast
    to bf16; applied additively to the score PSUM by the vector engine
    (one batched DMA per 512-query tile). exp() on the scalar engine.
    PSUM-reading copies are split across DVE (k/v, half the outputs) and
    ACT (q, the other half); gpsimd has no PSUM port.
  - The attention is a single flat software pipeline over (qt, kc) chunks:
    chunk t's P@V matmuls are emitted LAG chunks later, so the PE always
    has independent work in program order while the DVE mask-add + ACT exp
    of recent chunks are in flight. The pipeline runs straight across qt
    boundaries; the q-projection tiles for late query blocks are interleaved
    into early attention chunks as extra PE gap-fillers while their emb
    blocks stream in.
  - All input DMAs ride the SP ring in a hand-ordered sequence so the
    (serialized) DMA engines deliver each block just ahead of first use.
"""
import os

import numpy as np
import ml_dtypes

import concourse.bacc as bacc
import concourse.tile as tile
from concourse import mybir, bass2jax
from concourse.bass_utils import run_bass_kernel_spmd

# Debug aid (opt-in): surface real compile errors from the PJRT compile
# hook, which the C++ bridge otherwise swallows.
if os.environ.get("BASS_KERNEL_DEBUG"):
    import functools as _ft
    import traceback as _tb
    _orig_hook = bass2jax.neuronx_cc_hook
    @_ft.wraps(_orig_hook)
    def _dbg_hook(*args, **kwargs):
        try:
            return _orig_hook(*args, **kwargs)
        except BaseException:
            _tb.print_exc()
            raise
    bass2jax.neuronx_cc_hook = _dbg_hook

EMB, HID, B, L = 312, 256, 4, 4096
NCORES = 8
P = 128
KL = L // 2            # key rows per core (key-parallel halves)
EPAD = 384             # emb dim padded to 3 partition chunks; row EMB is the ones-row
HV = HID + 2           # v columns: HID values | ones | zero pad (even N)
QT = 512               # ql tile width (PSUM bank = 512 fp32)
NKC = KL // P          # 16 kl chunks per core
NQT = L // QT          # 8 ql tiles per core (all queries)
NEB = L // QT          # 8 emb column blocks
LAG = 3                # attention pipeline depth, in (qt, kc) chunks
N_WARM = 24            # PE clock-ramp warmup matmuls before the first input lands
MASK_SCALE = np.float32(-100000.0)

F32 = mybir.dt.float32
F32R = mybir.dt.float32r
BF16 = mybir.dt.bfloat16
BF = ml_dtypes.bfloat16

_CACHE = {}

# Leftover q-projection tiles (hc, block) interleaved into attention chunks
# (qt, kc) while their emb blocks are still streaming in.
_Q_INSERTS = {
    (0, 8): (0, 4), (0, 12): (1, 4),
    (1, 0): (0, 5), (1, 4): (1, 5), (1, 8): (0, 6), (1, 12): (1, 6),
    (2, 0): (0, 7), (2, 4): (1, 7),
}


def _build(precision="turbo"):
    nc = bacc.Bacc(None)

    embT = nc.dram_tensor("embT", [EPAD, L], F32R, kind="ExternalInput")
    wq = nc.dram_tensor("wq", [EPAD, HID], F32R, kind="ExternalInput")
    wk = nc.dram_tensor("wk", [EPAD, HID], F32R, kind="ExternalInput")
    wv = nc.dram_tensor("wv", [EPAD, HV], F32R, kind="ExternalInput")
    maskT = nc.dram_tensor("maskT", [KL, L], BF16, kind="ExternalInput")
    out = nc.dram_tensor("out", [L, HID + 1], F32, kind="ExternalOutput")

    with tile.TileContext(nc) as tc:
        with (
            tc.tile_pool(name="big", bufs=1) as big,
            tc.tile_pool(name="mt", bufs=3) as mtp,
            tc.tile_pool(name="pt", bufs=4) as ptp,
            tc.tile_pool(name="fin", bufs=2) as fin,
            tc.tile_pool(name="ps_st", bufs=4, space="PSUM") as ps_st,
            tc.tile_pool(name="ps_pv", bufs=1, space="PSUM") as ps_pv,
        ):
            emb_t = big.tile([P, 3, L], F32R, name="emb")
            wq_t = big.tile([P, 3, HID], F32R, name="wq")
            wk_t = big.tile([P, 3, HID], F32R, name="wk")
            wv_t = big.tile([P, 3, HV], F32R, name="wv")
            qT = big.tile([P, 2, L], F32R, name="qT")
            kT = big.tile([P, 2, KL], F32R, name="kT")
            v_r = big.tile([P, NKC, HV], F32R, name="v_r")

            # ---- DMA plan (all on the SP ring: issue order == transfer
            # order on the serialized DMA engines). Hand-ordered so each
            # block lands just ahead of its first PE use.
            def dma_cols(t, d, c0, c1):
                nc.sync.dma_start(
                    out=t[:, :, c0:c1],
                    in_=d[:, c0:c1].rearrange("(c p) n -> p c n", p=P),
                )

            def dma_emb(blk):
                dma_cols(emb_t, embT, blk * QT, (blk + 1) * QT)

            mts = [None] * NQT

            def dma_mask(qt, halves=False):
                mts[qt] = mtp.tile([P, NKC, QT], BF16, name="mt", tag="mt")
                qsl = slice(qt * QT, (qt + 1) * QT)
                src = maskT[:, qsl].rearrange("(c p) q -> p c q", p=P)
                if halves:
                    return (
                        lambda: nc.sync.dma_start(out=mts[qt][:, :NKC // 2], in_=src[:, :NKC // 2]),
                        lambda: nc.sync.dma_start(out=mts[qt][:, NKC // 2:], in_=src[:, NKC // 2:]),
                    )
                nc.sync.dma_start(out=mts[qt], in_=src)

            m0a, m0b = dma_mask(0, halves=True)
            m1a, m1b = dma_mask(1, halves=True)
            dma_cols(wk_t, wk, 0, P)        # first k-proj tile's stationary
            dma_cols(emb_t, embT, 0, P)     # first quarter of block 0
            dma_cols(emb_t, embT, P, 256)
            dma_cols(wk_t, wk, P, HID)
            dma_cols(emb_t, embT, 256, 512)
            dma_cols(wv_t, wv, 0, HV)
            dma_emb(1)
            dma_cols(wq_t, wq, 0, HID)
            dma_emb(2)
            m0a()
            dma_emb(3)
            m0b()
            dma_emb(4)
            m1a()
            m1b()
            dma_emb(5)
            dma_emb(6)
            dma_mask(2)
            dma_emb(7)

            # ---- PE clock-ramp warmup: the tensor engine needs ~3us of
            # continuous work to reach full clock. Chew on a memset tile
            # while the first input DMAs are still in flight so the real
            # matmuls start at speed.
            warm = big.tile([P, 256], BF16, name="warm")
            nc.gpsimd.memset(warm, 1.0)
            for _ in range(N_WARM):
                ps = ps_st.tile([P, QT], F32, name="st", tag="st")
                nc.tensor.matmul(
                    ps[:, :256], lhsT=warm[:, :P], rhs=warm, start=True, stop=True,
                )

            # ---- projections (single-pass fp32r, 3 e-chunk contraction)
            # q/k in [h(part), hc, l(free)] layout; v in [kl(part), klc, h].
            def emit_kq(hc, c0, c1, which):
                ps = ps_st.tile([P, QT], F32, name="st", tag="st")
                w, dst = (wk_t, kT) if which == "k" else (wq_t, qT)
                hsl = slice(hc * P, (hc + 1) * P)
                for e in range(3):
                    nc.tensor.matmul(
                        ps[:, :c1 - c0],
                        lhsT=w[:, e, hsl],
                        rhs=emb_t[:, e, c0:c1],
                        start=(e == 0), stop=(e == 2),
                    )
                # k copies on DVE, q copies on ACT (balance the engines).
                if which == "k":
                    nc.vector.tensor_copy(dst[:, hc, c0:c1], ps[:, :c1 - c0])
                else:
                    nc.scalar.copy(out=dst[:, hc, c0:c1], in_=ps[:, :c1 - c0])

            def emit_q(hc, blk):
                emit_kq(hc, blk * QT, (blk + 1) * QT, "q")

            def emit_v(kc):
                ps = ps_st.tile([P, QT], F32, name="st", tag="st")
                ksl = slice(kc * P, (kc + 1) * P)
                for e in range(3):
                    nc.tensor.matmul(
                        ps[:, :HV],
                        lhsT=emb_t[:, e, ksl],
                        rhs=wv_t[:, e, :],
                        start=(e == 0), stop=(e == 2),
                    )
                nc.vector.tensor_copy(v_r[:, kc, :], ps[:, :HV])

            # Block 0 in quarter-steps (its DMAs are split for fast start),
            # then blocks 1..3 with the early q tiles woven in. q tiles for
            # blocks 4..7 ride inside the attention stream (_Q_INSERTS).
            emit_kq(0, 0, P, "k")
            emit_kq(0, P, 256, "k")
            emit_kq(1, 0, 256, "k")
            emit_v(0)
            emit_v(1)
            emit_kq(0, 256, 512, "k")
            emit_kq(1, 256, 512, "k")
            emit_v(2)
            emit_v(3)
            for blk in range(1, 4):
                emit_kq(0, blk * QT, (blk + 1) * QT, "k")
                emit_kq(1, blk * QT, (blk + 1) * QT, "k")
                for i in range(4):
                    emit_v(4 * blk + i)
                emit_q(0, blk - 1)
                emit_q(1, blk - 1)
            emit_q(0, 3)
            emit_q(1, 3)

            # ---- attention: flat pipeline over 128 (qt, kc) chunks.
            pvs = None
            pend = []  # chunks whose P@V emission is deferred by LAG

            def flush_pv():
                qt, kc, pt, pv = pend.pop(0)
                for j in range(4):
                    jsl = slice(j * P, (j + 1) * P)
                    nc.tensor.matmul(
                        pv[j],
                        lhsT=pt[:, jsl],
                        rhs=v_r[:, kc, :],
                        start=(kc == 0), stop=(kc == NKC - 1),
                    )
                if kc == NKC - 1:
                    # Ship the unnormalized partial [sum p*v | sum p]; the
                    # host divides after combining the two key-halves. Copies
                    # split DVE/ACT; one batched out-DMA per qt on SP.
                    ot = fin.tile([P, 4, HID + 1], F32, name="ot", tag="ot")
                    for j in range(4):
                        src = pv[j][:, :HID + 1]
                        if j < 2:
                            nc.vector.tensor_copy(ot[:, j, :], src)
                        else:
                            nc.scalar.copy(out=ot[:, j, :], in_=src)
                    r0 = qt * QT
                    nc.sync.dma_start(
                        out=out[r0:r0 + QT, :].rearrange("(j p) h -> p j h", p=P),
                        in_=ot,
                    )

            for qt in range(NQT):
                if qt + 3 < NQT:
                    dma_mask(qt + 3)
                mt = mts[qt]
                pvs = [
                    ps_pv.tile([P, HV], F32, name=f"pv{j}", tag=f"pv{j}")
                    for j in range(4)
                ]
                qsl = slice(qt * QT, (qt + 1) * QT)
                for kc in range(NKC):
                    qi = _Q_INSERTS.get((qt, kc))
                    if qi is not None:
                        emit_q(*qi)
                    ksl = slice(kc * P, (kc + 1) * P)
                    st = ps_st.tile([P, QT], F32, name="st", tag="st")
                    for hc in range(2):
                        nc.tensor.matmul(
                            st,
                            lhsT=kT[:, hc, ksl],
                            rhs=qT[:, hc, qsl],
                            start=(hc == 0), stop=(hc == 1),
                        )
                    if len(pend) >= LAG:
                        flush_pv()
                    nc.vector.tensor_tensor(out=st, in0=st, in1=mt[:, kc, :], op=mybir.AluOpType.add)
                    pt = ptp.tile([P, QT], F32R, name="pt", tag="pt")
                    nc.scalar.activation(out=pt, in_=st, func=mybir.ActivationFunctionType.Exp)
                    pend.append((qt, kc, pt, pvs))
            while pend:
                flush_pv()
    nc.finalize()
    return nc


def _get_nc():
    precision = os.environ.get("BASS_KERNEL_PRECISION", "turbo")
    key = f"nc_{precision}"
    if key not in _CACHE:
        _CACHE[key] = _build(precision)
    return _CACHE[key]


def kernel(embedding, mask, Wq, bq, Wk, bk, Wv, bv):
    embedding = np.asarray(embedding, dtype=np.float32)
    mask = np.asarray(mask, dtype=np.float32)
    Wq = np.asarray(Wq, dtype=np.float32)
    Wk = np.asarray(Wk, dtype=np.float32)
    Wv = np.asarray(Wv, dtype=np.float32)
    bq = np.asarray(bq, dtype=np.float32)
    bk = np.asarray(bk, dtype=np.float32)
    bv = np.asarray(bv, dtype=np.float32)

    def pad_w(w, b, extra_one=False):
        wp = np.zeros((EPAD, HV if extra_one else HID), dtype=np.float32)
        wp[:EMB, :HID] = w
        wp[EMB, :HID] = b
        if extra_one:
            wp[EMB, HID] = 1.0
        return wp

    wq_p = pad_w(Wq, bq)
    wk_p = pad_w(Wk, bk)
    wv_p = pad_w(Wv, bv, extra_one=True)

    in_maps = []
    for c in range(NCORES):
        b, half = divmod(c, 2)
        # Rotate the sequence so this core's key half is rows 0..KL-1.
        emb_r = np.roll(embedding[b], -half * KL, axis=0)
        eT = np.zeros((EPAD, L), dtype=np.float32)
        eT[:EMB] = emb_r.T
        eT[EMB] = 1.0
        # maskT rows: this core's keys (original order); cols: rotated q.
        mT = np.roll(mask[b], -half * KL, axis=0)[:, half * KL:(half + 1) * KL].T
        mT = np.ascontiguousarray(mT * MASK_SCALE).astype(BF)
        in_maps.append({
            "embT": eT,
            "wq": wq_p, "wk": wk_p, "wv": wv_p,
            "maskT": mT,
        })

    nc = _get_nc()
    trace = bool(int(os.environ.get("BASS_KERNEL_TRACE", "0")))
    res = run_bass_kernel_spmd(nc, in_maps, core_ids=list(range(NCORES)), trace=trace)
    _CACHE["last_results"] = res

    full = np.empty((B, L, HID), dtype=np.float32)
    for b in range(B):
        r0 = res.results[2 * b]["out"].astype(np.float64)
        r1 = res.results[2 * b + 1]["out"].astype(np.float64)
        # r1 rows are in rotated q order (q = row + KL mod L); unroll.
        r1 = np.roll(r1, KL, axis=0)
        num = r0[:, :HID] + r1[:, :HID]
        den = r0[:, HID:] + r1[:, HID:]
        full[b] = (num / den).astype(np.float32)
    return full


# revision 16
# speedup vs baseline: 2.1458x; 1.0040x over previous
"""Single-head attention (B=4, L=4096, EMB=312, HID=256) on 8 NeuronCores.

Sharding: data-parallel over batch (4) x key-parallel (2) = 8 cores. Each
core handles ALL 4096 queries against its half of the keys and returns the
UNNORMALIZED partial [sum_k p*v | sum_k p] rows; the host combines the two
halves as (o1+o2)/(s1+s2).

Per-core device algorithm ("turbo"):
  - Every matmul is a single-pass fp32r op (1 row/cycle on the PE with a
    >=256-wide moving dim, near-fp32 accuracy). No hi/lo splits anywhere:
    projections contract 3 e-chunks, QK contracts 2 h-chunks, PV contracts
    16 key-chunks, all accumulating in PSUM.
  - The host rotates the embedding's sequence axis per-core so that the
    core's OWN key half sits in columns 0..2047; k/v projections then read
    a prefix of the same SBUF tile the q projection uses (no duplicate
    embTk load). Output rows come back in rotated order; the host unrolls.
  - embT carries a ones-row at index EMB and W* carry the bias in that row,
    so projections fold the bias in. Wv has 2 extra columns: ones (gives the
    softmax row-sum through the P@V matmul) and zero padding (even N).
  - Scores are computed transposed: sT[kl, ql] = kT-chunk^T @ qT, so the
    exp() output is directly the stationary operand for the P@V matmul -
    no on-device transposes anywhere.
  - Mask is host-side transposed/rotated and encoded as fp8 {0, -240}
    (exp underflows to exactly 0 either way, matching the reference);
    applied additively to the score PSUM by the vector engine (one batched
    DMA per 512-query tile). exp() on the scalar engine.
    PSUM-reading copies are split across DVE (k/v, half the outputs) and
    ACT (q, the other half); gpsimd has no PSUM port.
  - The attention is a single flat software pipeline over (qt, kc) chunks:
    chunk t's P@V matmuls are emitted LAG chunks later, so the PE always
    has independent work in program order while the DVE mask-add + ACT exp
    of recent chunks are in flight. The pipeline runs straight across qt
    boundaries; the q-projection tiles for late query blocks are interleaved
    into early attention chunks as extra PE gap-fillers while their emb
    blocks stream in.
  - All input DMAs ride the SP ring in a hand-ordered sequence so the
    (serialized) DMA engines deliver each block just ahead of first use.
"""
import os

import numpy as np
import ml_dtypes

import concourse.bacc as bacc
import concourse.tile as tile
from concourse import mybir, bass2jax
from concourse.bass_utils import run_bass_kernel_spmd

# Debug aid (opt-in): surface real compile errors from the PJRT compile
# hook, which the C++ bridge otherwise swallows.
if os.environ.get("BASS_KERNEL_DEBUG"):
    import functools as _ft
    import traceback as _tb
    _orig_hook = bass2jax.neuronx_cc_hook
    @_ft.wraps(_orig_hook)
    def _dbg_hook(*args, **kwargs):
        try:
            return _orig_hook(*args, **kwargs)
        except BaseException:
            _tb.print_exc()
            raise
    bass2jax.neuronx_cc_hook = _dbg_hook

EMB, HID, B, L = 312, 256, 4, 4096
NCORES = 8
P = 128
KL = L // 2            # key rows per core (key-parallel halves)
EPAD = 384             # emb dim padded to 3 partition chunks; row EMB is the ones-row
HV = HID + 2           # v columns: HID values | ones | zero pad (even N)
QT = 512               # ql tile width (PSUM bank = 512 fp32)
NKC = KL // P          # 16 kl chunks per core
NQT = L // QT          # 8 ql tiles per core (all queries)
NEB = L // QT          # 8 emb column blocks
LAG = 3                # attention pipeline depth, in (qt, kc) chunks
N_WARM = 13            # PE clock-ramp warmup matmuls before the first input lands
# Additive mask value. exp(score + MASK_SCALE) underflows fp32 to exactly 0
# for any reachable score (|score| < 40), matching the reference's -1e5 mask
# whose exp is also exactly 0 — and -240 is exactly representable in fp8.
MASK_SCALE = np.float32(-240.0)

F32 = mybir.dt.float32
F32R = mybir.dt.float32r
BF16 = mybir.dt.bfloat16
F8E4 = mybir.dt.float8e4
BF = ml_dtypes.bfloat16
F8 = ml_dtypes.float8_e4m3

_CACHE = {}

# Leftover q-projection tiles (hc, block) interleaved into attention chunks
# (qt, kc) while their emb blocks are still streaming in.
_Q_INSERTS = {
    (0, 8): (0, 4), (0, 12): (1, 4),
    (1, 0): (0, 5), (1, 4): (1, 5), (1, 8): (0, 6), (1, 12): (1, 6),
    (2, 0): (0, 7), (2, 4): (1, 7),
}


def _build(precision="turbo"):
    nc = bacc.Bacc(None)

    embT = nc.dram_tensor("embT", [EPAD, L], F32R, kind="ExternalInput")
    wq = nc.dram_tensor("wq", [EPAD, HID], F32R, kind="ExternalInput")
    wk = nc.dram_tensor("wk", [EPAD, HID], F32R, kind="ExternalInput")
    wv = nc.dram_tensor("wv", [EPAD, HV], F32R, kind="ExternalInput")
    maskT = nc.dram_tensor("maskT", [KL, L], F8E4, kind="ExternalInput")
    out = nc.dram_tensor("out", [L, HID + 1], F32, kind="ExternalOutput")

    with tile.TileContext(nc) as tc:
        with (
            tc.tile_pool(name="big", bufs=1) as big,
            tc.tile_pool(name="mt", bufs=3) as mtp,
            tc.tile_pool(name="pt", bufs=4) as ptp,
            tc.tile_pool(name="fin", bufs=2) as fin,
            tc.tile_pool(name="ps_st", bufs=4, space="PSUM") as ps_st,
            tc.tile_pool(name="ps_pv", bufs=1, space="PSUM") as ps_pv,
        ):
            emb_t = big.tile([P, 3, L], F32R, name="emb")
            wq_t = big.tile([P, 3, HID], F32R, name="wq")
            wk_t = big.tile([P, 3, HID], F32R, name="wk")
            wv_t = big.tile([P, 3, HV], F32R, name="wv")
            qT = big.tile([P, 2, L], F32R, name="qT")
            kT = big.tile([P, 2, KL], F32R, name="kT")
            v_r = big.tile([P, NKC, HV], F32R, name="v_r")

            # ---- DMA plan (all on the SP ring: issue order == transfer
            # order on the serialized DMA engines). Hand-ordered so each
            # block lands just ahead of its first PE use.
            def dma_cols(t, d, c0, c1):
                nc.sync.dma_start(
                    out=t[:, :, c0:c1],
                    in_=d[:, c0:c1].rearrange("(c p) n -> p c n", p=P),
                )

            def dma_emb(blk):
                dma_cols(emb_t, embT, blk * QT, (blk + 1) * QT)

            mts = [None] * NQT

            def dma_mask(qt):
                mts[qt] = mtp.tile([P, NKC, QT], F8E4, name="mt", tag="mt")
                qsl = slice(qt * QT, (qt + 1) * QT)
                nc.sync.dma_start(
                    out=mts[qt],
                    in_=maskT[:, qsl].rearrange("(c p) q -> p c q", p=P),
                )

            dma_cols(wk_t, wk, 0, P)        # first k-proj tile's stationary
            dma_cols(emb_t, embT, 0, 256)   # first half of block 0
            dma_cols(wk_t, wk, P, HID)
            dma_cols(wv_t, wv, 0, HV)
            dma_cols(emb_t, embT, 256, 512)
            dma_emb(1)
            dma_cols(wq_t, wq, 0, HID)
            dma_emb(2)
            dma_emb(3)
            dma_mask(0)
            dma_emb(4)
            dma_mask(1)
            dma_emb(5)
            dma_emb(6)
            dma_mask(2)
            dma_emb(7)

            # ---- PE clock-ramp warmup: the tensor engine needs ~3us of
            # continuous work to reach full clock. Chew on a memset tile
            # while the first input DMAs are still in flight so the real
            # matmuls start at speed; `warm_fill` plugs predicted DMA-wait
            # holes later in the projection phase to keep the clock pinned.
            warm = big.tile([P, 256], BF16, name="warm")
            nc.gpsimd.memset(warm, 1.0)

            def warm_fill(n):
                for _ in range(n):
                    ps = ps_st.tile([P, QT], F32, name="st", tag="st")
                    nc.tensor.matmul(
                        ps[:, :256], lhsT=warm[:, :P], rhs=warm, start=True, stop=True,
                    )

            warm_fill(N_WARM)

            # ---- projections (single-pass fp32r, 3 e-chunk contraction)
            # q/k in [h(part), hc, l(free)] layout; v in [kl(part), klc, h].
            def emit_kq(hc, c0, c1, which):
                ps = ps_st.tile([P, QT], F32, name="st", tag="st")
                w, dst = (wk_t, kT) if which == "k" else (wq_t, qT)
                hsl = slice(hc * P, (hc + 1) * P)
                for e in range(3):
                    nc.tensor.matmul(
                        ps[:, :c1 - c0],
                        lhsT=w[:, e, hsl],
                        rhs=emb_t[:, e, c0:c1],
                        start=(e == 0), stop=(e == 2),
                    )
                # k copies on DVE, q copies on ACT (balance the engines).
                if which == "k":
                    nc.vector.tensor_copy(dst[:, hc, c0:c1], ps[:, :c1 - c0])
                else:
                    nc.scalar.copy(out=dst[:, hc, c0:c1], in_=ps[:, :c1 - c0])

            def emit_q(hc, blk):
                emit_kq(hc, blk * QT, (blk + 1) * QT, "q")

            def emit_v(kc):
                ps = ps_st.tile([P, QT], F32, name="st", tag="st")
                ksl = slice(kc * P, (kc + 1) * P)
                for e in range(3):
                    nc.tensor.matmul(
                        ps[:, :HV],
                        lhsT=emb_t[:, e, ksl],
                        rhs=wv_t[:, e, :],
                        start=(e == 0), stop=(e == 2),
                    )
                nc.vector.tensor_copy(v_r[:, kc, :], ps[:, :HV])

            # Block 0 in half-steps (its DMAs are split for fast start),
            # then blocks 1..3 with the early q tiles woven in. q tiles for
            # blocks 4..7 ride inside the attention stream (_Q_INSERTS).
            # warm_fill between steps bridges predicted DMA-arrival holes.
            emit_kq(0, 0, 256, "k")
            warm_fill(2)
            emit_kq(1, 0, 256, "k")
            warm_fill(4)
            emit_v(0)
            emit_v(1)
            warm_fill(2)
            emit_kq(0, 256, 512, "k")
            emit_kq(1, 256, 512, "k")
            emit_v(2)
            emit_v(3)
            warm_fill(4)
            for blk in range(1, 4):
                emit_kq(0, blk * QT, (blk + 1) * QT, "k")
                emit_kq(1, blk * QT, (blk + 1) * QT, "k")
                for i in range(4):
                    emit_v(4 * blk + i)
                emit_q(0, blk - 1)
                emit_q(1, blk - 1)
            emit_q(0, 3)
            emit_q(1, 3)

            # ---- attention: flat pipeline over 128 (qt, kc) chunks.
            pvs = None
            pend = []  # chunks whose P@V emission is deferred by LAG

            def flush_pv():
                qt, kc, pt, pv = pend.pop(0)
                for j in range(4):
                    jsl = slice(j * P, (j + 1) * P)
                    nc.tensor.matmul(
                        pv[j],
                        lhsT=pt[:, jsl],
                        rhs=v_r[:, kc, :],
                        start=(kc == 0), stop=(kc == NKC - 1),
                    )
                if kc == NKC - 1:
                    # Ship the unnormalized partial [sum p*v | sum p]; the
                    # host divides after combining the two key-halves. Copies
                    # split DVE/ACT; one batched out-DMA per qt on SP.
                    ot = fin.tile([P, 4, HID + 1], F32, name="ot", tag="ot")
                    for j in range(4):
                        src = pv[j][:, :HID + 1]
                        if j < 2:
                            nc.vector.tensor_copy(ot[:, j, :], src)
                        else:
                            nc.scalar.copy(out=ot[:, j, :], in_=src)
                    r0 = qt * QT
                    nc.sync.dma_start(
                        out=out[r0:r0 + QT, :].rearrange("(j p) h -> p j h", p=P),
                        in_=ot,
                    )

            for qt in range(NQT):
                if qt + 3 < NQT:
                    dma_mask(qt + 3)
                mt = mts[qt]
                pvs = [
                    ps_pv.tile([P, HV], F32, name=f"pv{j}", tag=f"pv{j}")
                    for j in range(4)
                ]
                qsl = slice(qt * QT, (qt + 1) * QT)
                for kc in range(NKC):
                    qi = _Q_INSERTS.get((qt, kc))
                    if qi is not None:
                        emit_q(*qi)
                    ksl = slice(kc * P, (kc + 1) * P)
                    st = ps_st.tile([P, QT], F32, name="st", tag="st")
                    for hc in range(2):
                        nc.tensor.matmul(
                            st,
                            lhsT=kT[:, hc, ksl],
                            rhs=qT[:, hc, qsl],
                            start=(hc == 0), stop=(hc == 1),
                        )
                    if len(pend) >= LAG:
                        flush_pv()
                    nc.vector.tensor_tensor(out=st, in0=st, in1=mt[:, kc, :], op=mybir.AluOpType.add)
                    pt = ptp.tile([P, QT], F32R, name="pt", tag="pt")
                    nc.scalar.activation(out=pt, in_=st, func=mybir.ActivationFunctionType.Exp)
                    pend.append((qt, kc, pt, pvs))
            while pend:
                flush_pv()
    nc.finalize()
    return nc


def _get_nc():
    precision = os.environ.get("BASS_KERNEL_PRECISION", "turbo")
    key = f"nc_{precision}"
    if key not in _CACHE:
        _CACHE[key] = _build(precision)
    return _CACHE[key]


def kernel(embedding, mask, Wq, bq, Wk, bk, Wv, bv):
    embedding = np.asarray(embedding, dtype=np.float32)
    mask = np.asarray(mask, dtype=np.float32)
    Wq = np.asarray(Wq, dtype=np.float32)
    Wk = np.asarray(Wk, dtype=np.float32)
    Wv = np.asarray(Wv, dtype=np.float32)
    bq = np.asarray(bq, dtype=np.float32)
    bk = np.asarray(bk, dtype=np.float32)
    bv = np.asarray(bv, dtype=np.float32)

    def pad_w(w, b, extra_one=False):
        wp = np.zeros((EPAD, HV if extra_one else HID), dtype=np.float32)
        wp[:EMB, :HID] = w
        wp[EMB, :HID] = b
        if extra_one:
            wp[EMB, HID] = 1.0
        return wp

    wq_p = pad_w(Wq, bq)
    wk_p = pad_w(Wk, bk)
    wv_p = pad_w(Wv, bv, extra_one=True)

    in_maps = []
    for c in range(NCORES):
        b, half = divmod(c, 2)
        # Rotate the sequence so this core's key half is rows 0..KL-1.
        emb_r = np.roll(embedding[b], -half * KL, axis=0)
        eT = np.zeros((EPAD, L), dtype=np.float32)
        eT[:EMB] = emb_r.T
        eT[EMB] = 1.0
        # maskT rows: this core's keys (original order); cols: rotated q.
        mT = np.roll(mask[b], -half * KL, axis=0)[:, half * KL:(half + 1) * KL].T
        mT = np.ascontiguousarray(mT * MASK_SCALE).astype(F8)
        in_maps.append({
            "embT": eT,
            "wq": wq_p, "wk": wk_p, "wv": wv_p,
            "maskT": mT,
        })

    nc = _get_nc()
    trace = bool(int(os.environ.get("BASS_KERNEL_TRACE", "0")))
    res = run_bass_kernel_spmd(nc, in_maps, core_ids=list(range(NCORES)), trace=trace)
    _CACHE["last_results"] = res

    full = np.empty((B, L, HID), dtype=np.float32)
    for b in range(B):
        r0 = res.results[2 * b]["out"].astype(np.float64)
        r1 = res.results[2 * b + 1]["out"].astype(np.float64)
        # r1 rows are in rotated q order (q = row + KL mod L); unroll.
        r1 = np.roll(r1, KL, axis=0)
        num = r0[:, :HID] + r1[:, :HID]
        den = r0[:, HID:] + r1[:, HID:]
        full[b] = (num / den).astype(np.float32)
    return full


# revision 30
# speedup vs baseline: 2.1539x; 1.0038x over previous
"""Single-head attention (B=4, L=4096, EMB=312, HID=256) on 8 NeuronCores.

Sharding: data-parallel over batch (4) x key-parallel (2) = 8 cores. Each
core handles ALL 4096 queries against its half of the keys and returns the
UNNORMALIZED partial [sum_k p*v | sum_k p] rows; the host combines the two
halves as (o1+o2)/(s1+s2).

Per-core device algorithm ("turbo"):
  - Every matmul is a single-pass fp32r op (1 row/cycle on the PE with a
    >=256-wide moving dim, near-fp32 accuracy). No hi/lo splits anywhere:
    projections contract 3 e-chunks, QK contracts 2 h-chunks, PV contracts
    16 key-chunks, all accumulating in PSUM.
  - The host rotates the embedding's sequence axis per-core so that the
    core's OWN key half sits in columns 0..2047; k/v projections then read
    a prefix of the same SBUF tile the q projection uses (no duplicate
    embTk load). Output rows come back in rotated order; the host unrolls.
  - embT carries a ones-row at index EMB and W* carry the bias in that row,
    so projections fold the bias in. Wv has 2 extra columns: ones (gives the
    softmax row-sum through the P@V matmul) and zero padding (even N).
  - Scores are computed transposed: sT[kl, ql] = kT-chunk^T @ qT, so the
    exp() output is directly the stationary operand for the P@V matmul -
    no on-device transposes anywhere.
  - Mask is host-side transposed/rotated and encoded as fp8 {0, -240}
    (exp underflows to exactly 0 either way, matching the reference);
    applied additively to the score PSUM by the vector engine (one batched
    DMA per 512-query tile). exp() on the scalar engine.
    PSUM-reading copies are split across DVE (k/v, half the outputs) and
    ACT (q, the other half); gpsimd has no PSUM port.
  - The attention is a single flat software pipeline over (qt, kc) chunks:
    chunk t's P@V matmuls are emitted LAG chunks later, so the PE always
    has independent work in program order while the DVE mask-add + ACT exp
    of recent chunks are in flight. The pipeline runs straight across qt
    boundaries; the q-projection tiles for late query blocks are interleaved
    into early attention chunks as extra PE gap-fillers while their emb
    blocks stream in.
  - All input DMAs ride the SP ring in a hand-ordered sequence so the
    (serialized) DMA engines deliver each block just ahead of first use.
"""
import os

import numpy as np
import ml_dtypes

import concourse.bacc as bacc
import concourse.tile as tile
from concourse import mybir, bass2jax
from concourse.bass_utils import run_bass_kernel_spmd

# Debug aid (opt-in): surface real compile errors from the PJRT compile
# hook, which the C++ bridge otherwise swallows.
if os.environ.get("BASS_KERNEL_DEBUG"):
    import functools as _ft
    import traceback as _tb
    _orig_hook = bass2jax.neuronx_cc_hook
    @_ft.wraps(_orig_hook)
    def _dbg_hook(*args, **kwargs):
        try:
            return _orig_hook(*args, **kwargs)
        except BaseException:
            _tb.print_exc()
            raise
    bass2jax.neuronx_cc_hook = _dbg_hook

EMB, HID, B, L = 312, 256, 4, 4096
NCORES = 8
P = 128
KL = L // 2            # key rows per core (key-parallel halves)
EPAD = 384             # emb dim padded to 3 partition chunks; row EMB is the ones-row
HV = HID + 2           # v columns: HID values | ones | zero pad (even N)
QT = 512               # ql tile width (PSUM bank = 512 fp32)
NKC = KL // P          # 16 kl chunks per core
NQT = L // QT          # 8 ql tiles per core (all queries)
NEB = L // QT          # 8 emb column blocks
LAG = 4                # attention pipeline depth, in (qt, kc) chunks
N_WARM = 17            # PE clock-ramp warmup matmuls before the first input lands
# Additive mask value. exp(score + MASK_SCALE) underflows fp32 to exactly 0
# for any reachable score (|score| < 40), matching the reference's -1e5 mask
# whose exp is also exactly 0 — and -240 is exactly representable in fp8.
MASK_SCALE = np.float32(-240.0)

F32 = mybir.dt.float32
F32R = mybir.dt.float32r
BF16 = mybir.dt.bfloat16
F8E4 = mybir.dt.float8e4
BF = ml_dtypes.bfloat16
F8 = ml_dtypes.float8_e4m3

_CACHE = {}

# Leftover q-projection tiles (hc, block) interleaved into attention chunks
# (qt, kc) while their emb blocks are still streaming in.
_Q_INSERTS = {
    (0, 2): (0, 3), (0, 5): (1, 3),
    (0, 8): (0, 4), (0, 12): (1, 4),
    (1, 0): (0, 5), (1, 4): (1, 5), (1, 8): (0, 6), (1, 12): (1, 6),
    (2, 0): (0, 7), (2, 4): (1, 7),
}


def _build(precision="turbo"):
    nc = bacc.Bacc(None)

    embT = nc.dram_tensor("embT", [EPAD, L], F32R, kind="ExternalInput")
    wq = nc.dram_tensor("wq", [EPAD, HID], F32R, kind="ExternalInput")
    wk = nc.dram_tensor("wk", [EPAD, HID], F32R, kind="ExternalInput")
    wv = nc.dram_tensor("wv", [EPAD, HV], F32R, kind="ExternalInput")
    maskT = nc.dram_tensor("maskT", [KL, L], F8E4, kind="ExternalInput")
    out = nc.dram_tensor("out", [L, HID + 1], F32, kind="ExternalOutput")

    with tile.TileContext(nc) as tc:
        with (
            tc.tile_pool(name="big", bufs=1) as big,
            tc.tile_pool(name="mt", bufs=3) as mtp,
            tc.tile_pool(name="pt", bufs=5) as ptp,
            tc.tile_pool(name="fin", bufs=2) as fin,
            tc.tile_pool(name="ps_st", bufs=4, space="PSUM") as ps_st,
            tc.tile_pool(name="ps_pv", bufs=1, space="PSUM") as ps_pv,
        ):
            emb_t = big.tile([P, 3, L], F32R, name="emb")
            wq_t = big.tile([P, 3, HID], F32R, name="wq")
            wk_t = big.tile([P, 3, HID], F32R, name="wk")
            wv_t = big.tile([P, 3, HV], F32R, name="wv")
            qT = big.tile([P, 2, L], F32R, name="qT")
            kT = big.tile([P, 2, KL], F32R, name="kT")
            v_r = big.tile([P, NKC, HV], F32R, name="v_r")

            # ---- DMA plan (all on the SP ring: issue order == transfer
            # order on the serialized DMA engines). Hand-ordered so each
            # block lands just ahead of its first PE use.
            def dma_cols(t, d, c0, c1):
                nc.sync.dma_start(
                    out=t[:, :, c0:c1],
                    in_=d[:, c0:c1].rearrange("(c p) n -> p c n", p=P),
                )

            def dma_emb(blk):
                dma_cols(emb_t, embT, blk * QT, (blk + 1) * QT)

            mts = [None] * NQT

            def dma_mask(qt):
                mts[qt] = mtp.tile([P, NKC, QT], F8E4, name="mt", tag="mt")
                qsl = slice(qt * QT, (qt + 1) * QT)
                nc.sync.dma_start(
                    out=mts[qt],
                    in_=maskT[:, qsl].rearrange("(c p) q -> p c q", p=P),
                )

            dma_cols(wk_t, wk, 0, P)        # first k-proj tile's stationary
            dma_cols(emb_t, embT, 0, 256)   # first half of block 0
            dma_cols(wk_t, wk, P, HID)
            dma_cols(wv_t, wv, 0, HV)
            dma_cols(emb_t, embT, 256, 512)
            dma_emb(1)
            dma_cols(wq_t, wq, 0, HID)
            dma_emb(2)
            dma_emb(3)
            dma_mask(0)
            dma_emb(4)
            dma_mask(1)
            dma_emb(5)
            dma_emb(6)
            dma_mask(2)
            dma_emb(7)

            # ---- PE clock-ramp warmup: the tensor engine needs ~3us of
            # continuous work to reach full clock. Chew on a memset tile
            # while the first input DMAs are still in flight so the real
            # matmuls start at speed; `warm_fill` plugs predicted DMA-wait
            # holes later in the projection phase to keep the clock pinned.
            warm = big.tile([P, 256], BF16, name="warm")
            nc.vector.memset(warm, 1.0)

            def warm_fill(n):
                for _ in range(n):
                    ps = ps_st.tile([P, QT], F32, name="st", tag="st")
                    nc.tensor.matmul(
                        ps[:, :256], lhsT=warm[:, :P], rhs=warm, start=True, stop=True,
                    )

            warm_fill(N_WARM)

            # ---- projections (single-pass fp32r, 3 e-chunk contraction)
            # q/k in [h(part), hc, l(free)] layout; v in [kl(part), klc, h].
            def emit_kq(hc, c0, c1, which):
                ps = ps_st.tile([P, QT], F32, name="st", tag="st")
                w, dst = (wk_t, kT) if which == "k" else (wq_t, qT)
                hsl = slice(hc * P, (hc + 1) * P)
                for e in range(3):
                    nc.tensor.matmul(
                        ps[:, :c1 - c0],
                        lhsT=w[:, e, hsl],
                        rhs=emb_t[:, e, c0:c1],
                        start=(e == 0), stop=(e == 2),
                    )
                # k copies on DVE, q copies on ACT (balance the engines).
                if which == "k":
                    nc.vector.tensor_copy(dst[:, hc, c0:c1], ps[:, :c1 - c0])
                else:
                    nc.scalar.copy(out=dst[:, hc, c0:c1], in_=ps[:, :c1 - c0])

            def emit_q(hc, blk):
                emit_kq(hc, blk * QT, (blk + 1) * QT, "q")

            def emit_v(kc):
                ps = ps_st.tile([P, QT], F32, name="st", tag="st")
                ksl = slice(kc * P, (kc + 1) * P)
                for e in range(3):
                    nc.tensor.matmul(
                        ps[:, :HV],
                        lhsT=emb_t[:, e, ksl],
                        rhs=wv_t[:, e, :],
                        start=(e == 0), stop=(e == 2),
                    )
                nc.vector.tensor_copy(v_r[:, kc, :], ps[:, :HV])

            # Block 0 in half-steps (its DMAs are split for fast start),
            # then blocks 1..3 with the early q tiles woven in. q tiles for
            # blocks 4..7 ride inside the attention stream (_Q_INSERTS).
            # warm_fill between steps bridges predicted DMA-arrival holes.
            emit_kq(0, 0, 256, "k")
            warm_fill(3)
            emit_kq(1, 0, 256, "k")
            warm_fill(5)
            emit_v(0)
            emit_v(1)
            warm_fill(3)
            emit_kq(0, 256, 512, "k")
            emit_kq(1, 256, 512, "k")
            emit_v(2)
            emit_v(3)
            warm_fill(6)
            for blk in range(1, 4):
                emit_kq(0, blk * QT, (blk + 1) * QT, "k")
                emit_kq(1, blk * QT, (blk + 1) * QT, "k")
                for i in range(4):
                    emit_v(4 * blk + i)
                emit_q(0, blk - 1)
                emit_q(1, blk - 1)

            # ---- attention: flat pipeline over 128 (qt, kc) chunks.
            pvs = None
            pend = []  # chunks whose P@V emission is deferred by LAG

            def flush_pv():
                qt, kc, pt, pv = pend.pop(0)
                last = qt == NQT - 1 and kc == NKC - 1
                if last:
                    # Drain-tail special case: per-j PV + fin the moment each
                    # j's exp lands, and ship the output in two half-DMAs so
                    # the final transfer overlaps the remaining fins.
                    ot = fin.tile([P, 4, HID + 1], F32, name="ot", tag="ot")
                    r0 = qt * QT
                    for j in range(4):
                        jsl = slice(j * P, (j + 1) * P)
                        nc.tensor.matmul(
                            pv[j], lhsT=pt[:, jsl], rhs=v_r[:, kc, :],
                            start=False, stop=True,
                        )
                        if j % 2 == 0:
                            nc.vector.tensor_copy(ot[:, j, :], pv[j][:, :HID + 1])
                        else:
                            nc.scalar.copy(out=ot[:, j, :], in_=pv[j][:, :HID + 1])
                        if j % 2 == 1:
                            rj = r0 + (j - 1) * P
                            nc.sync.dma_start(
                                out=out[rj:rj + 2 * P, :].rearrange("(j p) h -> p j h", p=P),
                                in_=ot[:, j - 1:j + 1, :],
                            )
                    return
                for j in range(4):
                    jsl = slice(j * P, (j + 1) * P)
                    nc.tensor.matmul(
                        pv[j],
                        lhsT=pt[:, jsl],
                        rhs=v_r[:, kc, :],
                        start=(kc == 0), stop=(kc == NKC - 1),
                    )
                if kc == NKC - 1:
                    # Ship the unnormalized partial [sum p*v | sum p]; the
                    # host divides after combining the two key-halves. Copies
                    # split DVE/ACT; one batched out-DMA per qt on SP.
                    ot = fin.tile([P, 4, HID + 1], F32, name="ot", tag="ot")
                    for j in range(4):
                        src = pv[j][:, :HID + 1]
                        if j < 2:
                            nc.vector.tensor_copy(ot[:, j, :], src)
                        else:
                            nc.scalar.copy(out=ot[:, j, :], in_=src)
                    r0 = qt * QT
                    nc.sync.dma_start(
                        out=out[r0:r0 + QT, :].rearrange("(j p) h -> p j h", p=P),
                        in_=ot,
                    )

            for qt in range(NQT):
                if qt + 3 < NQT:
                    dma_mask(qt + 3)
                mt = mts[qt]
                pvs = [
                    ps_pv.tile([P, HV], F32, name=f"pv{j}", tag=f"pv{j}")
                    for j in range(4)
                ]
                qsl = slice(qt * QT, (qt + 1) * QT)
                for kc in range(NKC):
                    qi = _Q_INSERTS.get((qt, kc))
                    if qi is not None:
                        emit_q(*qi)
                    ksl = slice(kc * P, (kc + 1) * P)
                    st = ps_st.tile([P, QT], F32, name="st", tag="st")
                    for hc in range(2):
                        nc.tensor.matmul(
                            st,
                            lhsT=kT[:, hc, ksl],
                            rhs=qT[:, hc, qsl],
                            start=(hc == 0), stop=(hc == 1),
                        )
                    if len(pend) >= LAG:
                        flush_pv()
                    pt = ptp.tile([P, QT], F32R, name="pt", tag="pt")
                    if qt == NQT - 1 and kc == NKC - 1:
                        # Drain-tail: halve the mask-add and exp per j-slice
                        # so the first P@V can start ~1us sooner.
                        for h in range(2):
                            hs = slice(h * 256, (h + 1) * 256)
                            nc.vector.tensor_tensor(
                                out=st[:, hs], in0=st[:, hs], in1=mt[:, kc, hs],
                                op=mybir.AluOpType.add,
                            )
                        for j in range(4):
                            jsl = slice(j * P, (j + 1) * P)
                            nc.scalar.activation(
                                out=pt[:, jsl], in_=st[:, jsl],
                                func=mybir.ActivationFunctionType.Exp,
                            )
                    else:
                        nc.vector.tensor_tensor(out=st, in0=st, in1=mt[:, kc, :], op=mybir.AluOpType.add)
                        nc.scalar.activation(out=pt, in_=st, func=mybir.ActivationFunctionType.Exp)
                    pend.append((qt, kc, pt, pvs))
            while pend:
                flush_pv()
    nc.finalize()
    return nc


def _get_nc():
    precision = os.environ.get("BASS_KERNEL_PRECISION", "turbo")
    key = f"nc_{precision}"
    if key not in _CACHE:
        _CACHE[key] = _build(precision)
    return _CACHE[key]


def kernel(embedding, mask, Wq, bq, Wk, bk, Wv, bv):
    embedding = np.asarray(embedding, dtype=np.float32)
    mask = np.asarray(mask, dtype=np.float32)
    Wq = np.asarray(Wq, dtype=np.float32)
    Wk = np.asarray(Wk, dtype=np.float32)
    Wv = np.asarray(Wv, dtype=np.float32)
    bq = np.asarray(bq, dtype=np.float32)
    bk = np.asarray(bk, dtype=np.float32)
    bv = np.asarray(bv, dtype=np.float32)

    def pad_w(w, b, extra_one=False):
        wp = np.zeros((EPAD, HV if extra_one else HID), dtype=np.float32)
        wp[:EMB, :HID] = w
        wp[EMB, :HID] = b
        if extra_one:
            wp[EMB, HID] = 1.0
        return wp

    wq_p = pad_w(Wq, bq)
    wk_p = pad_w(Wk, bk)
    wv_p = pad_w(Wv, bv, extra_one=True)

    in_maps = []
    for c in range(NCORES):
        b, half = divmod(c, 2)
        # Rotate the sequence so this core's key half is rows 0..KL-1.
        emb_r = np.roll(embedding[b], -half * KL, axis=0)
        eT = np.zeros((EPAD, L), dtype=np.float32)
        eT[:EMB] = emb_r.T
        eT[EMB] = 1.0
        # maskT rows: this core's keys (original order); cols: rotated q.
        mT = np.roll(mask[b], -half * KL, axis=0)[:, half * KL:(half + 1) * KL].T
        mT = np.ascontiguousarray(mT * MASK_SCALE).astype(F8)
        in_maps.append({
            "embT": eT,
            "wq": wq_p, "wk": wk_p, "wv": wv_p,
            "maskT": mT,
        })

    nc = _get_nc()
    trace = bool(int(os.environ.get("BASS_KERNEL_TRACE", "0")))
    res = run_bass_kernel_spmd(nc, in_maps, core_ids=list(range(NCORES)), trace=trace)
    _CACHE["last_results"] = res

    full = np.empty((B, L, HID), dtype=np.float32)
    for b in range(B):
        r0 = res.results[2 * b]["out"].astype(np.float64)
        r1 = res.results[2 * b + 1]["out"].astype(np.float64)
        # r1 rows are in rotated q order (q = row + KL mod L); unroll.
        r1 = np.roll(r1, KL, axis=0)
        num = r0[:, :HID] + r1[:, :HID]
        den = r0[:, HID:] + r1[:, HID:]
        full[b] = (num / den).astype(np.float32)
    return full


# revision 41
# speedup vs baseline: 2.1717x; 1.0083x over previous
"""Single-head attention (B=4, L=4096, EMB=312, HID=256) on 8 NeuronCores.

Sharding: data-parallel over batch (4) x key-parallel (2) = 8 cores. Each
core handles ALL 4096 queries against its half of the keys and returns the
UNNORMALIZED partial [sum_k p*v | sum_k p] rows; the host combines the two
halves as (o1+o2)/(s1+s2).

Per-core device algorithm ("turbo"):
  - Every matmul is a single-pass fp32r op (1 row/cycle on the PE with a
    >=256-wide moving dim, near-fp32 accuracy). No hi/lo splits anywhere:
    projections contract 3 e-chunks, QK contracts 2 h-chunks, PV contracts
    16 key-chunks, all accumulating in PSUM.
  - The host rotates the embedding's sequence axis per-core so that the
    core's OWN key half sits in columns 0..2047; k/v projections then read
    a prefix of the same SBUF tile the q projection uses (no duplicate
    embTk load). Output rows come back in rotated order; the host unrolls.
  - embT carries a ones-row at index EMB and W* carry the bias in that row,
    so projections fold the bias in. Wv has 2 extra columns: ones (gives the
    softmax row-sum through the P@V matmul) and zero padding (even N).
  - Scores are computed transposed: sT[kl, ql] = kT-chunk^T @ qT, so the
    exp() output is directly the stationary operand for the P@V matmul -
    no on-device transposes anywhere.
  - Mask is host-side transposed/rotated and encoded as fp8 {0, -240}
    (exp underflows to exactly 0 either way, matching the reference);
    applied additively to the score PSUM by the vector engine (one batched
    DMA per 512-query tile). exp() on the scalar engine.
    PSUM-reading copies are split across DVE (k/v, half the outputs) and
    ACT (q, the other half); gpsimd has no PSUM port.
  - The attention is a single flat software pipeline over (qt, kc) chunks:
    chunk t's P@V matmuls are emitted LAG chunks later, so the PE always
    has independent work in program order while the DVE mask-add + ACT exp
    of recent chunks are in flight. The pipeline runs straight across qt
    boundaries; the q-projection tiles for late query blocks are interleaved
    into early attention chunks as extra PE gap-fillers while their emb
    blocks stream in.
  - All input DMAs ride the SP ring in a hand-ordered sequence so the
    (serialized) DMA engines deliver each block just ahead of first use.
"""
import os

import numpy as np
import ml_dtypes

import concourse.bacc as bacc
import concourse.tile as tile
from concourse import mybir, bass2jax
from concourse.bass_utils import run_bass_kernel_spmd

# Debug aid (opt-in): surface real compile errors from the PJRT compile
# hook, which the C++ bridge otherwise swallows.
if os.environ.get("BASS_KERNEL_DEBUG"):
    import functools as _ft
    import traceback as _tb
    _orig_hook = bass2jax.neuronx_cc_hook
    @_ft.wraps(_orig_hook)
    def _dbg_hook(*args, **kwargs):
        try:
            return _orig_hook(*args, **kwargs)
        except BaseException:
            _tb.print_exc()
            raise
    bass2jax.neuronx_cc_hook = _dbg_hook

EMB, HID, B, L = 312, 256, 4, 4096
NCORES = 8
P = 128
KL = L // 2            # key rows per core (key-parallel halves)
EPAD = 384             # emb dim padded to 3 partition chunks; row EMB is the ones-row
HV = HID + 2           # v columns: HID values | ones | zero pad (even N)
QT = 512               # ql tile width (PSUM bank = 512 fp32)
NKC = KL // P          # 16 kl chunks per core
NQT = L // QT          # 8 ql tiles per core (all queries)
NEB = L // QT          # 8 emb column blocks
LAG = 4                # attention pipeline depth, in (qt, kc) chunks
N_WARM = 14            # PE clock-ramp warmup matmuls before the first input lands
# Additive mask value. exp(score + MASK_SCALE) underflows fp32 to exactly 0
# for any reachable score (|score| < 40), matching the reference's -1e5 mask
# whose exp is also exactly 0 — and -240 is exactly representable in fp8.
MASK_SCALE = np.float32(-240.0)

F32 = mybir.dt.float32
F32R = mybir.dt.float32r
F16 = mybir.dt.float16
BF16 = mybir.dt.bfloat16
F8E4 = mybir.dt.float8e4
BF = ml_dtypes.bfloat16
F8 = ml_dtypes.float8_e4m3

_CACHE = {}

# Leftover q-projection tiles (hc, block) interleaved into attention chunks
# (qt, kc) while their emb blocks are still streaming in.
_Q_INSERTS = {
    (0, 2): (0, 3), (0, 5): (1, 3),
    (0, 8): (0, 4), (0, 12): (1, 4),
    (1, 0): (0, 5), (1, 4): (1, 5), (1, 8): (0, 6), (1, 12): (1, 6),
    (2, 0): (0, 7), (2, 4): (1, 7),
}


def _build(precision="turbo"):
    nc = bacc.Bacc(None)

    embT = nc.dram_tensor("embT", [EPAD, L], F16, kind="ExternalInput")
    wq = nc.dram_tensor("wq", [EPAD, HID], F16, kind="ExternalInput")
    wk = nc.dram_tensor("wk", [EPAD, HID], F16, kind="ExternalInput")
    wv = nc.dram_tensor("wv", [EPAD, HV], F16, kind="ExternalInput")
    maskT = nc.dram_tensor("maskT", [KL, L], F8E4, kind="ExternalInput")
    out = nc.dram_tensor("out", [L, HID + 1], F32, kind="ExternalOutput")

    with tile.TileContext(nc) as tc:
        with (
            tc.tile_pool(name="big", bufs=1) as big,
            tc.tile_pool(name="mt", bufs=3) as mtp,
            tc.tile_pool(name="pt", bufs=5) as ptp,
            tc.tile_pool(name="fin", bufs=2) as fin,
            tc.tile_pool(name="ps_st", bufs=4, space="PSUM") as ps_st,
            tc.tile_pool(name="ps_pv", bufs=1, space="PSUM") as ps_pv,
        ):
            emb_t = big.tile([P, 3, L], F16, name="emb")
            wq_t = big.tile([P, 3, HID], F16, name="wq")
            wk_t = big.tile([P, 3, HID], F16, name="wk")
            wv_t = big.tile([P, 3, HV], F16, name="wv")
            qT = big.tile([P, 2, L], F32R, name="qT")
            kT = big.tile([P, 2, KL], F32R, name="kT")
            v_r = big.tile([P, NKC, HV], F32R, name="v_r")

            # ---- DMA plan (all on the SP ring: issue order == transfer
            # order on the serialized DMA engines). Hand-ordered so each
            # block lands just ahead of its first PE use.
            def dma_cols(t, d, c0, c1):
                nc.sync.dma_start(
                    out=t[:, :, c0:c1],
                    in_=d[:, c0:c1].rearrange("(c p) n -> p c n", p=P),
                )

            def dma_emb(blk):
                dma_cols(emb_t, embT, blk * QT, (blk + 1) * QT)

            mts = [None] * NQT

            def dma_mask(qt):
                mts[qt] = mtp.tile([P, NKC, QT], F8E4, name="mt", tag="mt")
                qsl = slice(qt * QT, (qt + 1) * QT)
                nc.sync.dma_start(
                    out=mts[qt],
                    in_=maskT[:, qsl].rearrange("(c p) q -> p c q", p=P),
                )

            dma_cols(wk_t, wk, 0, P)        # first k-proj tile's stationary
            dma_cols(emb_t, embT, 0, 256)   # first half of block 0
            dma_cols(wk_t, wk, P, HID)
            dma_cols(wv_t, wv, 0, HV)
            dma_cols(emb_t, embT, 256, 512)
            dma_emb(1)
            dma_cols(wq_t, wq, 0, HID)
            dma_emb(2)
            dma_emb(3)
            dma_mask(0)
            dma_emb(4)
            dma_mask(1)
            dma_emb(5)
            dma_emb(6)
            dma_mask(2)
            dma_emb(7)

            # ---- PE clock-ramp warmup: the tensor engine needs ~3us of
            # continuous work to reach full clock. Chew on a memset tile
            # while the first input DMAs are still in flight so the real
            # matmuls start at speed; `warm_fill` plugs predicted DMA-wait
            # holes later in the projection phase to keep the clock pinned.
            warm = big.tile([P, 256], BF16, name="warm")
            nc.gpsimd.memset(warm, 1.0)

            def warm_fill(n):
                for _ in range(n):
                    ps = ps_st.tile([P, QT], F32, name="st", tag="st")
                    nc.tensor.matmul(
                        ps[:, :256], lhsT=warm[:, :P], rhs=warm, start=True, stop=True,
                    )

            warm_fill(N_WARM)

            # ---- projections (single-pass fp32r, 3 e-chunk contraction)
            # q/k in [h(part), hc, l(free)] layout; v in [kl(part), klc, h].
            def emit_kq(hc, c0, c1, which):
                ps = ps_st.tile([P, QT], F32, name="st", tag="st")
                w, dst = (wk_t, kT) if which == "k" else (wq_t, qT)
                hsl = slice(hc * P, (hc + 1) * P)
                for e in range(3):
                    nc.tensor.matmul(
                        ps[:, :c1 - c0],
                        lhsT=w[:, e, hsl],
                        rhs=emb_t[:, e, c0:c1],
                        start=(e == 0), stop=(e == 2),
                    )
                # k copies on DVE, q copies on ACT (balance the engines).
                if which == "k":
                    nc.vector.tensor_copy(dst[:, hc, c0:c1], ps[:, :c1 - c0])
                else:
                    nc.scalar.copy(out=dst[:, hc, c0:c1], in_=ps[:, :c1 - c0])

            def emit_q(hc, blk):
                emit_kq(hc, blk * QT, (blk + 1) * QT, "q")

            def emit_v(kc):
                ps = ps_st.tile([P, QT], F32, name="st", tag="st")
                ksl = slice(kc * P, (kc + 1) * P)
                for e in range(3):
                    nc.tensor.matmul(
                        ps[:, :HV],
                        lhsT=emb_t[:, e, ksl],
                        rhs=wv_t[:, e, :],
                        start=(e == 0), stop=(e == 2),
                    )
                nc.vector.tensor_copy(v_r[:, kc, :], ps[:, :HV])

            # Block 0 in half-steps (its DMAs are split for fast start),
            # then blocks 1..3 with the early q tiles woven in. q tiles for
            # blocks 4..7 ride inside the attention stream (_Q_INSERTS).
            # warm_fill between steps bridges predicted DMA-arrival holes.
            emit_kq(0, 0, 256, "k")
            warm_fill(1)
            emit_kq(1, 0, 256, "k")
            warm_fill(5)
            emit_v(0)
            emit_v(1)
            warm_fill(3)
            emit_kq(0, 256, 512, "k")
            emit_kq(1, 256, 512, "k")
            emit_v(2)
            emit_v(3)
            warm_fill(4)
            for blk in range(1, 4):
                emit_kq(0, blk * QT, (blk + 1) * QT, "k")
                emit_kq(1, blk * QT, (blk + 1) * QT, "k")
                for i in range(4):
                    emit_v(4 * blk + i)
                emit_q(0, blk - 1)
                emit_q(1, blk - 1)

            # ---- attention: flat pipeline over 128 (qt, kc) chunks.
            pvs = None
            pend = []  # chunks whose P@V emission is deferred by LAG

            def flush_pv():
                qt, kc, pt, pv = pend.pop(0)
                if qt == NQT - 1 and kc == NKC - 1:
                    # Drain-tail special case: per-j PV + fin the moment each
                    # j's exp lands, and ship the output in two half-DMAs so
                    # the final transfer overlaps the remaining fins.
                    ot = fin.tile([P, 4, HID + 1], F32, name="ot", tag="ot")
                    r0 = qt * QT
                    for j in range(4):
                        jsl = slice(j * P, (j + 1) * P)
                        nc.tensor.matmul(
                            pv[j], lhsT=pt[:, jsl], rhs=v_r[:, kc, :],
                            start=False, stop=True,
                        )
                        if j % 2 == 0:
                            nc.vector.tensor_copy(ot[:, j, :], pv[j][:, :HID + 1])
                        else:
                            nc.scalar.copy(out=ot[:, j, :], in_=pv[j][:, :HID + 1])
                        if j % 2 == 1:
                            rj = r0 + (j - 1) * P
                            nc.sync.dma_start(
                                out=out[rj:rj + 2 * P, :].rearrange("(j p) h -> p j h", p=P),
                                in_=ot[:, j - 1:j + 1, :],
                            )
                    return
                for j in range(4):
                    jsl = slice(j * P, (j + 1) * P)
                    nc.tensor.matmul(
                        pv[j],
                        lhsT=pt[:, jsl],
                        rhs=v_r[:, kc, :],
                        start=(kc == 0), stop=(kc == NKC - 1),
                    )
                if kc == NKC - 1:
                    # Ship the unnormalized partial [sum p*v | sum p]; the
                    # host divides after combining the two key-halves. Copies
                    # split DVE/ACT; one batched out-DMA per qt on SP.
                    ot = fin.tile([P, 4, HID + 1], F32, name="ot", tag="ot")
                    for j in range(4):
                        src = pv[j][:, :HID + 1]
                        if j < 2:
                            nc.vector.tensor_copy(ot[:, j, :], src)
                        else:
                            nc.scalar.copy(out=ot[:, j, :], in_=src)
                    r0 = qt * QT
                    nc.sync.dma_start(
                        out=out[r0:r0 + QT, :].rearrange("(j p) h -> p j h", p=P),
                        in_=ot,
                    )

            for qt in range(NQT):
                if qt + 3 < NQT:
                    dma_mask(qt + 3)
                mt = mts[qt]
                pvs = [
                    ps_pv.tile([P, HV], F32, name=f"pv{j}", tag=f"pv{j}")
                    for j in range(4)
                ]
                qsl = slice(qt * QT, (qt + 1) * QT)
                for kc in range(NKC):
                    qi = _Q_INSERTS.get((qt, kc))
                    if qi is not None:
                        emit_q(*qi)
                    ksl = slice(kc * P, (kc + 1) * P)
                    st = ps_st.tile([P, QT], F32, name="st", tag="st")
                    for hc in range(2):
                        nc.tensor.matmul(
                            st,
                            lhsT=kT[:, hc, ksl],
                            rhs=qT[:, hc, qsl],
                            start=(hc == 0), stop=(hc == 1),
                        )
                    if len(pend) >= LAG:
                        flush_pv()
                    pt = ptp.tile([P, QT], F32R, name="pt", tag="pt")
                    if qt == NQT - 1 and kc == NKC - 1:
                        # Drain-tail: halve the mask-add and exp per j-slice
                        # so each trailing P@V can start sooner.
                        for h in range(2):
                            hs = slice(h * 256, (h + 1) * 256)
                            nc.vector.tensor_tensor(
                                out=st[:, hs], in0=st[:, hs], in1=mt[:, kc, hs],
                                op=mybir.AluOpType.add,
                            )
                        for j in range(4):
                            jsl = slice(j * P, (j + 1) * P)
                            nc.scalar.activation(
                                out=pt[:, jsl], in_=st[:, jsl],
                                func=mybir.ActivationFunctionType.Exp,
                            )
                    else:
                        nc.vector.tensor_tensor(out=st, in0=st, in1=mt[:, kc, :], op=mybir.AluOpType.add)
                        nc.scalar.activation(out=pt, in_=st, func=mybir.ActivationFunctionType.Exp)
                    pend.append((qt, kc, pt, pvs))
            while pend:
                flush_pv()
    nc.finalize()
    return nc


def _get_nc():
    precision = os.environ.get("BASS_KERNEL_PRECISION", "turbo")
    key = f"nc_{precision}"
    if key not in _CACHE:
        _CACHE[key] = _build(precision)
    return _CACHE[key]


def kernel(embedding, mask, Wq, bq, Wk, bk, Wv, bv):
    embedding = np.asarray(embedding, dtype=np.float32)
    mask = np.asarray(mask, dtype=np.float32)
    Wq = np.asarray(Wq, dtype=np.float32)
    Wk = np.asarray(Wk, dtype=np.float32)
    Wv = np.asarray(Wv, dtype=np.float32)
    bq = np.asarray(bq, dtype=np.float32)
    bk = np.asarray(bk, dtype=np.float32)
    bv = np.asarray(bv, dtype=np.float32)

    def pad_w(w, b, extra_one=False):
        wp = np.zeros((EPAD, HV if extra_one else HID), dtype=np.float32)
        wp[:EMB, :HID] = w
        wp[EMB, :HID] = b
        if extra_one:
            wp[EMB, HID] = 1.0
        return wp

    wq_p = pad_w(Wq, bq)
    wk_p = pad_w(Wk, bk)
    wv_p = pad_w(Wv, bv, extra_one=True)

    in_maps = []
    for c in range(NCORES):
        b, half = divmod(c, 2)
        # Rotate the sequence so this core's key half is rows 0..KL-1.
        emb_r = np.roll(embedding[b], -half * KL, axis=0)
        eT = np.zeros((EPAD, L), dtype=np.float32)
        eT[:EMB] = emb_r.T
        eT[EMB] = 1.0
        # maskT rows: this core's keys (original order); cols: rotated q.
        mT = np.roll(mask[b], -half * KL, axis=0)[:, half * KL:(half + 1) * KL].T
        mT = np.ascontiguousarray(mT * MASK_SCALE).astype(F8)
        in_maps.append({
            "embT": eT.astype(np.float16),
            "wq": wq_p.astype(np.float16),
            "wk": wk_p.astype(np.float16),
            "wv": wv_p.astype(np.float16),
            "maskT": mT,
        })

    nc = _get_nc()
    trace = bool(int(os.environ.get("BASS_KERNEL_TRACE", "0")))
    res = run_bass_kernel_spmd(nc, in_maps, core_ids=list(range(NCORES)), trace=trace)
    _CACHE["last_results"] = res

    full = np.empty((B, L, HID), dtype=np.float32)
    for b in range(B):
        r0 = res.results[2 * b]["out"].astype(np.float64)
        r1 = res.results[2 * b + 1]["out"].astype(np.float64)
        # r1 rows are in rotated q order (q = row + KL mod L); unroll.
        r1 = np.roll(r1, KL, axis=0)
        num = r0[:, :HID] + r1[:, :HID]
        den = r0[:, HID:] + r1[:, HID:]
        full[b] = (num / den).astype(np.float32)
    return full
